# revision 14
# baseline (speedup 1.0000x reference)
"""Causal self-attention Trainium2 kernel (8 NeuronCores, tensor-parallel heads).

Problem: B=4, T=2048, C=1024, H=16, D=64 fp32.
  q,k,v = x@Wq+bq, x@Wk+bk, x@Wv+bv  (per-head causal softmax(qk^T/8) @ v) @ Wp + bp

Sharding: 2 heads per core (column-split Wq/Wk/Wv, row-split Wp). Each core
computes a partial output [B*T, C]; host sums the 8 partials and adds bp.

Per-core dataflow (all matmuls in fp32r: full PE rate at N>=512, ~1e-4 rel err):
  xT [C, B*T] fp32 streamed from DRAM (host pre-transposes x).
  Q^T/K^T/V^T [128, B*T] = w.T @ xT  (w slices [C,128] as stationary).
  V natural [tk,64] per head via PE transpose of V^T tiles; ones column
  appended -> V_aug [tk, 65] so P@V_aug also yields softmax row sums.
  S^T [tk,512] = K^T.T @ Q^T per (batch, tq-block, tk-tile); both heads run
  concurrently as K=64 matmuls on distinct PE row groups into one 2-bank
  PSUM tile. Causality is a second accumulated matmul adding -50 on masked
  entries (L/E triangular factorization); softmax then skips the
  max-subtraction pass entirely (scores are O(1); exp stays in fp32 range).
  P^T = exp(S^T) via one double-wide ACT op per slot, PSUM -> SBUF f32r.
  Y_aug^T [65, 512] = V_aug.T @ P^T accumulated over tk tiles; row 64 is the
  softmax denominator. Normalize: reciprocal_approx_fast -> gpsimd
  partition_broadcast -> DVE multiply -> Y^T [128, B*T].
  out_part [512, 1024] = Y^T.T @ Wp_slice per block, DMA'd out as computed.
"""

import numpy as np

import concourse.bass as bass
import concourse.tile as tile
from concourse import bacc, mybir
from concourse.bass_utils import run_bass_kernel_spmd

F32R = mybir.dt.float32r
F32 = mybir.dt.float32
BF16 = mybir.dt.bfloat16

B, T, C, H = 4, 2048, 1024, 16
D = C // H  # 64
N_CORES = 8
RT = 512  # row-tile (tq block) size
KT = C // 128  # 8 contraction tiles for projections
NEG = -50.0  # causal mask additive constant (exp(-50+s) ~ 1e-20)


def build_kernel(n_batches=B):
    nc = bacc.Bacc(None, target_bir_lowering=False, debug=False)
    rows = n_batches * T
    bt_rt = T // RT  # 4 tq blocks per batch

    xT_d = nc.dram_tensor("xT", [C, rows], F32R, kind="ExternalInput")
    wq_d = nc.dram_tensor("wq", [C, 128], F32R, kind="ExternalInput")
    wk_d = nc.dram_tensor("wk", [C, 128], F32R, kind="ExternalInput")
    wv_d = nc.dram_tensor("wv", [C, 128], F32R, kind="ExternalInput")
    wp_d = nc.dram_tensor("wp", [128, C], F32R, kind="ExternalInput")
    bq_d = nc.dram_tensor("bq", [128], F32, kind="ExternalInput")
    bk_d = nc.dram_tensor("bk", [128], F32, kind="ExternalInput")
    bv_d = nc.dram_tensor("bv", [128], F32, kind="ExternalInput")
    L_d = nc.dram_tensor("Lm", [128, 128], BF16, kind="ExternalInput")
    E_d = nc.dram_tensor("Em", [128, 4, RT], BF16, kind="ExternalInput")
    id_d = nc.dram_tensor("ident", [128, 64], F32R, kind="ExternalInput")
    on_d = nc.dram_tensor("onescol", [128, 2 * (T // 128)], F32R, kind="ExternalInput")
    out_d = nc.dram_tensor("out", [rows, C], F32, kind="ExternalOutput")

    with tile.TileContext(nc) as tc:
        with (
            nc.allow_low_precision(reason="f32r intermediates are intentional"),
            tc.tile_pool(name="const", bufs=1) as const,
            tc.tile_pool(name="big", bufs=1) as big,
            tc.tile_pool(name="xs", bufs=3) as xs,
            tc.tile_pool(name="vt", bufs=3) as vtp,
            tc.tile_pool(name="pt", bufs=3) as ptp,
            tc.tile_pool(name="yt", bufs=4) as ytp,
            tc.tile_pool(name="nrm", bufs=3) as nrm,
            tc.tile_pool(name="ob", bufs=3) as ob,
            # PSUM: tag "s" slots are [128, 2, RT] = 2 banks (S^T both heads;
            # projection accumulators use half a slot). bufs=2 -> 4 banks.
            # Tag "y" slots are 1 bank, shared by Y accumulators, out-proj
            # tiles and V-transpose tiles. bufs=4 -> 4 banks. Total 8.
            tc.tile_pool(name="psS", bufs=2, space="PSUM") as psS,
            tc.tile_pool(name="psY", bufs=4, space="PSUM") as psY,
        ):
            # ---- constants ----
            wq = const.tile([128, KT, 128], F32R)
            wk = const.tile([128, KT, 128], F32R)
            wv = const.tile([128, KT, 128], F32R)
            wp = const.tile([128, C], F32R)
            nc.sync.dma_start(wq[:], wq_d.rearrange("(k p) m -> p k m", p=128))
            nc.sync.dma_start(wk[:], wk_d.rearrange("(k p) m -> p k m", p=128))
            nc.sync.dma_start(wv[:], wv_d.rearrange("(k p) m -> p k m", p=128))
            biases = []
            for name, d in (("bq", bq_d), ("bk", bk_d), ("bv", bv_d)):
                t = const.tile([128, 1], F32, name=f"{name}_sb")
                nc.gpsimd.dma_start(t[:], d.rearrange("(p o) -> p o", o=1))
                biases.append(t)
            Lm = const.tile([128, 128], BF16)
            Em = const.tile([128, 4, RT], BF16)
            ident = const.tile([128, 64], F32R)
            nc.gpsimd.dma_start(ident[:], id_d[:])
            nc.gpsimd.dma_start(Lm[:], L_d[:])
            nc.gpsimd.dma_start(Em[:], E_d[:])
            nc.gpsimd.dma_start(wp[:], wp_d[:])

            # ---- whole-run big buffers (per-rowtile tiles so attention
            # blocks only depend on the projection rowtiles they read) ----
            n_rt_all = rows // RT
            qTs = [big.tile([128, RT], F32R, name=f"qT{i}") for i in range(n_rt_all)]
            kTs = [big.tile([128, RT], F32R, name=f"kT{i}") for i in range(n_rt_all)]
            n_vt = T // 128  # 16 v-tiles per batch per head
            v_aug = big.tile([128, 2, n_vt, 65], F32R)  # per-batch, reused
            nc.gpsimd.dma_start(
                v_aug[:, :, :, 64:65],
                on_d.rearrange("p (h t o) -> p h t o", h=2, o=1),
            )

            for b in range(n_batches):
                r0 = b * T
                # ---- projections for batch b: 4 row-tiles of 512 ----
                with nc.named_scope(f"proj{b}"):
                    for rt in range(bt_rt):
                        c0 = r0 + rt * RT
                        xt = xs.tile([128, KT, RT], F32R, name="xt")
                        x_src = xT_d.rearrange("(k p) r -> p k r", p=128)
                        for kh in range(0, KT, 2):
                            nc.sync.dma_start(
                                xt[:, kh : kh + 2, :],
                                x_src[:, kh : kh + 2, c0 : c0 + RT],
                            )
                        for w, bias, dest in (
                            (wq, biases[0], qTs[(r0 + rt * RT) // RT]),
                            (wk, biases[1], kTs[(r0 + rt * RT) // RT]),
                            (wv, biases[2], None),
                        ):
                            acc = psS.tile([128, RT], F32, name="proj", tag="s")
                            for k in range(KT):
                                nc.tensor.matmul(
                                    acc[:],
                                    w[:, k, :],
                                    xt[:, k, :],
                                    start=(k == 0),
                                    stop=(k == KT - 1),
                                )
                            if dest is not None:
                                nc.vector.tensor_scalar_add(
                                    dest[:], acc[:], bias[:]
                                )
                            else:
                                vt_sb = vtp.tile([128, RT], F32R, name="vt_sb")
                                nc.vector.tensor_scalar_add(vt_sb[:], acc[:], bias[:])
                                for c in range(RT // 128):
                                    vtile = rt * (RT // 128) + c
                                    vps = psY.tile(
                                        [128, 2, 64], F32R, name="vps", tag="y"
                                    )
                                    for h in range(2):
                                        nc.tensor.transpose(
                                            vps[:, h, :],
                                            vt_sb[
                                                64 * h : 64 * h + 64,
                                                c * 128 : c * 128 + 128,
                                            ],
                                            ident[64 * h : 64 * h + 64, :],
                                        )
                                        nc.vector.tensor_copy(
                                            v_aug[:, h, vtile, 0:64], vps[:, h, :]
                                        )

                # ---- attention for batch b ----
                for tqb in range(bt_rt):
                    with nc.named_scope(f"attn{b}_{tqb}"):
                        q0 = r0 + tqb * RT
                        n_tk = (tqb + 1) * (RT // 128)
                        yps = [
                            psY.tile([65, RT], F32, name=f"yacc{h}", tag="y")
                            for h in range(2)
                        ]
                        for tk in range(n_tk):
                            k0 = r0 + tk * 128
                            diag = tk * 128 >= tqb * RT
                            st = psS.tile([128, 2, RT], F32, name="st", tag="s")
                            kt_tile = kTs[k0 // RT]
                            kk = k0 % RT
                            qt_tile = qTs[q0 // RT]
                            for h in range(2):
                                hs = slice(64 * h, 64 * h + 64)
                                nc.tensor.matmul(
                                    st[:, h, :],
                                    kt_tile[hs, kk : kk + 128],
                                    qt_tile[hs, :],
                                    start=True,
                                    stop=not diag,
                                    skip_group_check=True,
                                )
                            if diag:
                                off_idx = tk - tqb * (RT // 128)
                                for h in range(2):
                                    nc.tensor.matmul(
                                        st[:, h, :],
                                        Lm[:],
                                        Em[:, off_idx, :],
                                        start=False,
                                        stop=True,
                                        skip_group_check=True,
                                    )
                            pt = ptp.tile([128, 2, RT], F32R, name="pt")
                            nc.scalar.activation(
                                pt[:], st[:], mybir.ActivationFunctionType.Exp
                            )
                            for h in range(2):
                                nc.tensor.matmul(
                                    yps[h][:],
                                    v_aug[:, h, tk, :],
                                    pt[:, h, :],
                                    start=(tk == 0),
                                    stop=(tk == n_tk - 1),
                                )
                        # ---- normalize -> Y^T block [128, RT] ----
                        yt = ytp.tile([128, RT], F32R, name="yt")
                        for h in range(2):
                            ssum = nrm.tile([1, RT], F32, name="ssum")
                            nc.vector.tensor_copy(ssum[:], yps[h][64:65, :])
                            srow = nrm.tile([1, RT], F32, name="srow")
                            nc.vector.reciprocal_approx_fast(srow[:], ssum[:])
                            bc = nrm.tile([64, RT], F32, name="bc")
                            nc.gpsimd.partition_broadcast(bc[:], srow[:])
                            nc.vector.tensor_mul(
                                yt[64 * h : 64 * h + 64, :], yps[h][0:64, :], bc[:]
                            )
                    # ---- output projection for this 512-row block ----
                    with nc.named_scope(f"oproj{b}_{tqb}"):
                        for rr in range(RT // 128):
                            for nn in range(C // 512):
                                ops = psY.tile([128, 512], F32, name="ops", tag="y")
                                nc.tensor.matmul(
                                    ops[:],
                                    yt[:, rr * 128 : rr * 128 + 128],
                                    wp[:, nn * 512 : nn * 512 + 512],
                                    start=True,
                                    stop=True,
                                )
                                osb = ob.tile([128, 512], F32, name="osb")
                                nc.vector.tensor_copy(osb[:], ops[:])
                                nc.sync.dma_start(
                                    out_d[
                                        q0 + rr * 128 : q0 + rr * 128 + 128,
                                        nn * 512 : nn * 512 + 512,
                                    ],
                                    osb[:],
                                )
    nc.compile()
    return nc


def make_masks():
    """L/E such that (L.T @ E)[i, j] = NEG iff masked (key i+tk0 > query j+tq0,
    given off = tk0 - tq0 in {0,128,256,384}), else 0."""
    L = np.zeros((128, 128), np.float32)
    for k in range(128):
        L[k, k + 1 :] = NEG
    L[127, :] = NEG
    E = np.zeros((128, 4, RT), np.float32)
    for oi, off in enumerate((0, 128, 256, 384)):
        for k in range(127):
            if k + off < RT:
                E[k, oi, k + off] = 1.0
        E[127, oi, :off] = 1.0
    return L, E


def make_inputs_for_core(c, x, Wq, bq, Wk, bk, Wv, bv, Wp, bp, n_batches=B):
    cols = slice(c * 128, (c + 1) * 128)
    xT = np.ascontiguousarray(
        np.asarray(x, np.float32).reshape(B * T, C)[: n_batches * T].T
    )
    L, E = make_masks()
    idnp = np.zeros((128, 64), np.float32)
    for h in range(2):
        idnp[64 * h : 64 * h + 64] = np.eye(64, dtype=np.float32)
    import ml_dtypes

    return {
        "xT": xT,
        "wq": np.ascontiguousarray(np.asarray(Wq, np.float32)[:, cols] / 8.0),
        "wk": np.ascontiguousarray(np.asarray(Wk, np.float32)[:, cols]),
        "wv": np.ascontiguousarray(np.asarray(Wv, np.float32)[:, cols]),
        "wp": np.ascontiguousarray(np.asarray(Wp, np.float32)[cols, :]),
        "bq": np.ascontiguousarray(np.asarray(bq, np.float32)[cols] / 8.0),
        "bk": np.ascontiguousarray(np.asarray(bk, np.float32)[cols]),
        "bv": np.ascontiguousarray(np.asarray(bv, np.float32)[cols]),
        "Lm": L.astype(ml_dtypes.bfloat16),
        "Em": E.astype(ml_dtypes.bfloat16),
        "ident": idnp,
        "onescol": np.ones((128, 2 * (T // 128)), np.float32),
    }


def kernel(x, Wq, bq, Wk, bk, Wv, bv, Wp, bp, _nc_cache={}, **run_kwargs):
    n_batches = B
    if "nc" not in _nc_cache:
        _nc_cache["nc"] = build_kernel(n_batches)
    nc = _nc_cache["nc"]
    in_maps = [
        make_inputs_for_core(c, x, Wq, bq, Wk, bk, Wv, bv, Wp, bp, n_batches)
        for c in range(N_CORES)
    ]
    res = run_bass_kernel_spmd(nc, in_maps, core_ids=list(range(N_CORES)), **run_kwargs)
    out = np.zeros((B * T, C), np.float32)
    for r in res.results:
        out += r["out"]
    out += np.asarray(bp, np.float32)[None, :]
    if run_kwargs.get("trace"):
        kernel.last_result = res
    return out.reshape(B, T, C)


# revision 15
# speedup vs baseline: 1.1849x; 1.1849x over previous
"""Causal self-attention Trainium2 kernel (8 NeuronCores, tensor-parallel heads).

Problem: B=4, T=2048, C=1024, H=16, D=64 fp32.
  q,k,v = x@Wq+bq, x@Wk+bk, x@Wv+bv  (per-head causal softmax(qk^T/8) @ v) @ Wp + bp

Sharding: 2 heads per core (column-split Wq/Wk/Wv, row-split Wp). Each core
computes a partial output [B*T, C]; host sums the 8 partials and adds bp.

Per-core dataflow (all matmuls in fp32r: full PE rate at N>=512, ~1e-4 rel err):
  xT [C, B*T] fp32 streamed from DRAM (host pre-transposes x).
  Q^T/K^T/V^T [128, B*T] = w.T @ xT  (w slices [C,128] as stationary).
  V natural [tk,64] per head via PE transpose of V^T tiles; ones column
  appended -> V_aug [tk, 65] so P@V_aug also yields softmax row sums.
  S^T [tk,512] = K^T.T @ Q^T per (batch, tq-block, tk-tile); both heads run
  concurrently as K=64 matmuls on distinct PE row groups into one 2-bank
  PSUM tile. Causality is a second accumulated matmul adding -50 on masked
  entries (L/E triangular factorization); softmax then skips the
  max-subtraction pass entirely (scores are O(1); exp stays in fp32 range).
  P^T = exp(S^T) via one double-wide ACT op per slot, PSUM -> SBUF f32r.
  Y_aug^T [65, 512] = V_aug.T @ P^T accumulated over tk tiles; row 64 is the
  softmax denominator. Normalize: reciprocal_approx_fast -> gpsimd
  partition_broadcast -> DVE multiply -> Y^T [128, B*T].
  out_part [512, 1024] = Y^T.T @ Wp_slice per block, DMA'd out as computed.
"""

import numpy as np

import concourse.bass as bass
import concourse.tile as tile
from concourse import bacc, mybir
from concourse.bass_utils import run_bass_kernel_spmd

F32R = mybir.dt.float32r
F32 = mybir.dt.float32
BF16 = mybir.dt.bfloat16

B, T, C, H = 4, 2048, 1024, 16
D = C // H  # 64
N_CORES = 8
RT = 512  # row-tile (tq block) size
KT = C // 128  # 8 contraction tiles for projections
NEG = -50.0  # causal mask additive constant (exp(-50+s) ~ 1e-20)


def build_kernel(n_batches=B):
    nc = bacc.Bacc(None, target_bir_lowering=False, debug=False)
    rows = n_batches * T
    bt_rt = T // RT  # 4 tq blocks per batch

    xT_d = nc.dram_tensor("xT", [C, rows], F32R, kind="ExternalInput")
    wq_d = nc.dram_tensor("wq", [C, 128], F32R, kind="ExternalInput")
    wk_d = nc.dram_tensor("wk", [C, 128], F32R, kind="ExternalInput")
    wv_d = nc.dram_tensor("wv", [C, 128], F32R, kind="ExternalInput")
    wp_d = nc.dram_tensor("wp", [128, C], F32R, kind="ExternalInput")
    bq_d = nc.dram_tensor("bq", [128], F32, kind="ExternalInput")
    bk_d = nc.dram_tensor("bk", [128], F32, kind="ExternalInput")
    bv_d = nc.dram_tensor("bv", [128], F32, kind="ExternalInput")
    L_d = nc.dram_tensor("Lm", [128, 128], BF16, kind="ExternalInput")
    E_d = nc.dram_tensor("Em", [128, 4, RT], BF16, kind="ExternalInput")
    id_d = nc.dram_tensor("ident", [128, 64], F32R, kind="ExternalInput")
    on_d = nc.dram_tensor("onescol", [128, 2 * (T // 128)], F32R, kind="ExternalInput")
    out_d = nc.dram_tensor("out", [rows, C], F32, kind="ExternalOutput")

    with tile.TileContext(nc) as tc:
        with (
            nc.allow_low_precision(reason="f32r intermediates are intentional"),
            tc.tile_pool(name="const", bufs=1) as const,
            tc.tile_pool(name="big", bufs=1) as big,
            tc.tile_pool(name="xs", bufs=3) as xs,
            tc.tile_pool(name="vt", bufs=3) as vtp,
            tc.tile_pool(name="pt", bufs=3) as ptp,
            tc.tile_pool(name="yt", bufs=4) as ytp,
            tc.tile_pool(name="nrm", bufs=3) as nrm,
            tc.tile_pool(name="ob", bufs=3) as ob,
            # PSUM: tag "s" slots are [128, 2, RT] = 2 banks (S^T both heads;
            # projection accumulators use half a slot). bufs=2 -> 4 banks.
            # Tag "y" slots are 1 bank, shared by Y accumulators, out-proj
            # tiles and V-transpose tiles. bufs=4 -> 4 banks. Total 8.
            tc.tile_pool(name="psS", bufs=2, space="PSUM") as psS,
            tc.tile_pool(name="psY", bufs=3, space="PSUM") as psY,
            tc.tile_pool(name="psO", bufs=1, space="PSUM") as psO,
        ):
            # ---- constants ----
            wq = const.tile([128, KT, 128], F32R)
            wk = const.tile([128, KT, 128], F32R)
            wv = const.tile([128, KT, 128], F32R)
            wp = const.tile([128, C], F32R)
            nc.sync.dma_start(wq[:], wq_d.rearrange("(k p) m -> p k m", p=128))
            nc.sync.dma_start(wk[:], wk_d.rearrange("(k p) m -> p k m", p=128))
            nc.sync.dma_start(wv[:], wv_d.rearrange("(k p) m -> p k m", p=128))
            biases = []
            for name, d in (("bq", bq_d), ("bk", bk_d), ("bv", bv_d)):
                t = const.tile([128, 1], F32, name=f"{name}_sb")
                nc.gpsimd.dma_start(t[:], d.rearrange("(p o) -> p o", o=1))
                biases.append(t)
            Lm = const.tile([128, 128], BF16)
            Em = const.tile([128, 4, RT], BF16)
            ident = const.tile([128, 64], F32R)
            nc.gpsimd.dma_start(ident[:], id_d[:])
            nc.gpsimd.dma_start(Lm[:], L_d[:])
            nc.gpsimd.dma_start(Em[:], E_d[:])
            nc.gpsimd.dma_start(wp[:], wp_d[:])

            # ---- whole-run big buffers (per-rowtile tiles so attention
            # blocks only depend on the projection rowtiles they read) ----
            n_rt_all = rows // RT
            qTs = [big.tile([128, RT], F32R, name=f"qT{i}") for i in range(n_rt_all)]
            kTs = [big.tile([128, RT], F32R, name=f"kT{i}") for i in range(n_rt_all)]
            n_vt = T // 128  # 16 v-tiles per batch per head
            v_aug = big.tile([128, 2, n_vt, 65], F32R)  # per-batch, reused
            nc.gpsimd.dma_start(
                v_aug[:, :, :, 64:65],
                on_d.rearrange("p (h t o) -> p h t o", h=2, o=1),
            )

            for b in range(n_batches):
                r0 = b * T
                # ---- projections for batch b: 4 row-tiles of 512 ----
                with nc.named_scope(f"proj{b}"):
                    for rt in range(bt_rt):
                        c0 = r0 + rt * RT
                        xt = xs.tile([128, KT, RT], F32R, name="xt")
                        x_src = xT_d.rearrange("(k p) r -> p k r", p=128)
                        for kh in range(0, KT, 2):
                            nc.sync.dma_start(
                                xt[:, kh : kh + 2, :],
                                x_src[:, kh : kh + 2, c0 : c0 + RT],
                            )
                        for w, bias, dest in (
                            (wq, biases[0], qTs[(r0 + rt * RT) // RT]),
                            (wk, biases[1], kTs[(r0 + rt * RT) // RT]),
                            (wv, biases[2], None),
                        ):
                            acc = psS.tile([128, RT], F32, name="proj", tag="s")
                            for k in range(KT):
                                nc.tensor.matmul(
                                    acc[:],
                                    w[:, k, :],
                                    xt[:, k, :],
                                    start=(k == 0),
                                    stop=(k == KT - 1),
                                )
                            if dest is not None:
                                nc.vector.tensor_scalar_add(
                                    dest[:], acc[:], bias[:]
                                )
                            else:
                                vt_sb = vtp.tile([128, RT], F32R, name="vt_sb")
                                nc.vector.tensor_scalar_add(vt_sb[:], acc[:], bias[:])
                                for c in range(RT // 128):
                                    vtile = rt * (RT // 128) + c
                                    vps = psO.tile(
                                        [128, 2, 64], F32R, name="vps", tag="o"
                                    )
                                    for h in range(2):
                                        nc.tensor.transpose(
                                            vps[:, h, :],
                                            vt_sb[
                                                64 * h : 64 * h + 64,
                                                c * 128 : c * 128 + 128,
                                            ],
                                            ident[64 * h : 64 * h + 64, :],
                                        )
                                        nc.vector.tensor_copy(
                                            v_aug[:, h, vtile, 0:64], vps[:, h, :]
                                        )

                # ---- attention for batch b ----
                for tqb in range(bt_rt):
                    with nc.named_scope(f"attn{b}_{tqb}"):
                        q0 = r0 + tqb * RT
                        n_tk = (tqb + 1) * (RT // 128)
                        yps = [
                            psY.tile([65, RT], F32, name=f"yacc{h}", tag="y")
                            for h in range(2)
                        ]
                        for tk in range(n_tk):
                            k0 = r0 + tk * 128
                            diag = tk * 128 >= tqb * RT
                            st = psS.tile([128, 2, RT], F32, name="st", tag="s")
                            kt_tile = kTs[k0 // RT]
                            kk = k0 % RT
                            qt_tile = qTs[q0 // RT]
                            for h in range(2):
                                hs = slice(64 * h, 64 * h + 64)
                                nc.tensor.matmul(
                                    st[:, h, :],
                                    kt_tile[hs, kk : kk + 128],
                                    qt_tile[hs, :],
                                    start=True,
                                    stop=not diag,
                                    skip_group_check=True,
                                )
                            if diag:
                                off_idx = tk - tqb * (RT // 128)
                                for h in range(2):
                                    nc.tensor.matmul(
                                        st[:, h, :],
                                        Lm[:],
                                        Em[:, off_idx, :],
                                        start=False,
                                        stop=True,
                                        skip_group_check=True,
                                    )
                            pt = ptp.tile([128, 2, RT], F32R, name="pt")
                            nc.scalar.activation(
                                pt[:], st[:], mybir.ActivationFunctionType.Exp
                            )
                            for h in range(2):
                                nc.tensor.matmul(
                                    yps[h][:],
                                    v_aug[:, h, tk, :],
                                    pt[:, h, :],
                                    start=(tk == 0),
                                    stop=(tk == n_tk - 1),
                                )
                        # ---- normalize -> Y^T block [128, RT] ----
                        yt = ytp.tile([128, RT], F32R, name="yt")
                        for h in range(2):
                            ssum = nrm.tile([1, RT], F32, name="ssum")
                            nc.vector.tensor_copy(ssum[:], yps[h][64:65, :])
                            srow = nrm.tile([1, RT], F32, name="srow")
                            nc.vector.reciprocal_approx_fast(srow[:], ssum[:])
                            bc = nrm.tile([64, RT], F32, name="bc")
                            nc.gpsimd.partition_broadcast(bc[:], srow[:])
                            nc.vector.tensor_mul(
                                yt[64 * h : 64 * h + 64, :], yps[h][0:64, :], bc[:]
                            )
                    # ---- output projection for this 512-row block ----
                    with nc.named_scope(f"oproj{b}_{tqb}"):
                        for rr in range(RT // 128):
                            for nn in range(C // 512):
                                ops = psO.tile([128, 512], F32, name="ops", tag="o")
                                nc.tensor.matmul(
                                    ops[:],
                                    yt[:, rr * 128 : rr * 128 + 128],
                                    wp[:, nn * 512 : nn * 512 + 512],
                                    start=True,
                                    stop=True,
                                )
                                osb = ob.tile([128, 512], F32, name="osb")
                                nc.vector.tensor_copy(osb[:], ops[:])
                                nc.sync.dma_start(
                                    out_d[
                                        q0 + rr * 128 : q0 + rr * 128 + 128,
                                        nn * 512 : nn * 512 + 512,
                                    ],
                                    osb[:],
                                )
    nc.compile()
    return nc


def make_masks():
    """L/E such that (L.T @ E)[i, j] = NEG iff masked (key i+tk0 > query j+tq0,
    given off = tk0 - tq0 in {0,128,256,384}), else 0."""
    L = np.zeros((128, 128), np.float32)
    for k in range(128):
        L[k, k + 1 :] = NEG
    L[127, :] = NEG
    E = np.zeros((128, 4, RT), np.float32)
    for oi, off in enumerate((0, 128, 256, 384)):
        for k in range(127):
            if k + off < RT:
                E[k, oi, k + off] = 1.0
        E[127, oi, :off] = 1.0
    return L, E


def make_inputs_for_core(c, x, Wq, bq, Wk, bk, Wv, bv, Wp, bp, n_batches=B):
    cols = slice(c * 128, (c + 1) * 128)
    xT = np.ascontiguousarray(
        np.asarray(x, np.float32).reshape(B * T, C)[: n_batches * T].T
    )
    L, E = make_masks()
    idnp = np.zeros((128, 64), np.float32)
    for h in range(2):
        idnp[64 * h : 64 * h + 64] = np.eye(64, dtype=np.float32)
    import ml_dtypes

    return {
        "xT": xT,
        "wq": np.ascontiguousarray(np.asarray(Wq, np.float32)[:, cols] / 8.0),
        "wk": np.ascontiguousarray(np.asarray(Wk, np.float32)[:, cols]),
        "wv": np.ascontiguousarray(np.asarray(Wv, np.float32)[:, cols]),
        "wp": np.ascontiguousarray(np.asarray(Wp, np.float32)[cols, :]),
        "bq": np.ascontiguousarray(np.asarray(bq, np.float32)[cols] / 8.0),
        "bk": np.ascontiguousarray(np.asarray(bk, np.float32)[cols]),
        "bv": np.ascontiguousarray(np.asarray(bv, np.float32)[cols]),
        "Lm": L.astype(ml_dtypes.bfloat16),
        "Em": E.astype(ml_dtypes.bfloat16),
        "ident": idnp,
        "onescol": np.ones((128, 2 * (T // 128)), np.float32),
    }


def kernel(x, Wq, bq, Wk, bk, Wv, bv, Wp, bp, _nc_cache={}, **run_kwargs):
    n_batches = B
    if "nc" not in _nc_cache:
        _nc_cache["nc"] = build_kernel(n_batches)
    nc = _nc_cache["nc"]
    in_maps = [
        make_inputs_for_core(c, x, Wq, bq, Wk, bk, Wv, bv, Wp, bp, n_batches)
        for c in range(N_CORES)
    ]
    res = run_bass_kernel_spmd(nc, in_maps, core_ids=list(range(N_CORES)), **run_kwargs)
    out = np.zeros((B * T, C), np.float32)
    for r in res.results:
        out += r["out"]
    out += np.asarray(bp, np.float32)[None, :]
    if run_kwargs.get("trace"):
        kernel.last_result = res
    return out.reshape(B, T, C)


# revision 16
# speedup vs baseline: 1.1923x; 1.0063x over previous
"""Causal self-attention Trainium2 kernel (8 NeuronCores, tensor-parallel heads).

Problem: B=4, T=2048, C=1024, H=16, D=64 fp32.
  q,k,v = x@Wq+bq, x@Wk+bk, x@Wv+bv  (per-head causal softmax(qk^T/8) @ v) @ Wp + bp

Sharding: 2 heads per core (column-split Wq/Wk/Wv, row-split Wp). Each core
computes a partial output [B*T, C]; host sums the 8 partials and adds bp.

Per-core dataflow (all matmuls in fp32r: full PE rate at N>=512, ~1e-4 rel err):
  xT [C, B*T] fp32 streamed from DRAM (host pre-transposes x).
  Q^T/K^T/V^T [128, B*T] = w.T @ xT  (w slices [C,128] as stationary).
  V natural [tk,64] per head via PE transpose of V^T tiles; ones column
  appended -> V_aug [tk, 65] so P@V_aug also yields softmax row sums.
  S^T [tk,512] = K^T.T @ Q^T per (batch, tq-block, tk-tile); both heads run
  concurrently as K=64 matmuls on distinct PE row groups into one 2-bank
  PSUM tile. Causality is a second accumulated matmul adding -50 on masked
  entries (L/E triangular factorization); softmax then skips the
  max-subtraction pass entirely (scores are O(1); exp stays in fp32 range).
  P^T = exp(S^T) via one double-wide ACT op per slot, PSUM -> SBUF f32r.
  Y_aug^T [65, 512] = V_aug.T @ P^T accumulated over tk tiles; row 64 is the
  softmax denominator. Normalize: reciprocal_approx_fast -> gpsimd
  partition_broadcast -> DVE multiply -> Y^T [128, B*T].
  out_part [512, 1024] = Y^T.T @ Wp_slice per block, DMA'd out as computed.
"""

import numpy as np

import concourse.bass as bass
import concourse.tile as tile
from concourse import bacc, mybir
from concourse.bass_utils import run_bass_kernel_spmd

F32R = mybir.dt.float32r
F32 = mybir.dt.float32
BF16 = mybir.dt.bfloat16

B, T, C, H = 4, 2048, 1024, 16
D = C // H  # 64
N_CORES = 8
RT = 512  # row-tile (tq block) size
KT = C // 128  # 8 contraction tiles for projections
NEG = -50.0  # causal mask additive constant (exp(-50+s) ~ 1e-20)


def build_kernel(n_batches=B):
    nc = bacc.Bacc(None, target_bir_lowering=False, debug=False)
    rows = n_batches * T
    bt_rt = T // RT  # 4 tq blocks per batch

    xT_d = nc.dram_tensor("xT", [C, rows], F32R, kind="ExternalInput")
    wq_d = nc.dram_tensor("wq", [C, 128], F32R, kind="ExternalInput")
    wk_d = nc.dram_tensor("wk", [C, 128], F32R, kind="ExternalInput")
    wv_d = nc.dram_tensor("wv", [C, 128], F32R, kind="ExternalInput")
    wp_d = nc.dram_tensor("wp", [128, C], F32R, kind="ExternalInput")
    bq_d = nc.dram_tensor("bq", [128], F32, kind="ExternalInput")
    bk_d = nc.dram_tensor("bk", [128], F32, kind="ExternalInput")
    bv_d = nc.dram_tensor("bv", [128], F32, kind="ExternalInput")
    L_d = nc.dram_tensor("Lm", [128, 128], BF16, kind="ExternalInput")
    E_d = nc.dram_tensor("Em", [128, 4, RT], BF16, kind="ExternalInput")
    id_d = nc.dram_tensor("ident", [128, 64], F32R, kind="ExternalInput")
    on_d = nc.dram_tensor("onescol", [128, 2 * (T // 128)], F32R, kind="ExternalInput")
    out_d = nc.dram_tensor("out", [rows, C], F32, kind="ExternalOutput")

    with tile.TileContext(nc) as tc:
        with (
            nc.allow_low_precision(reason="f32r intermediates are intentional"),
            tc.tile_pool(name="const", bufs=1) as const,
            tc.tile_pool(name="big", bufs=1) as big,
            tc.tile_pool(name="xs", bufs=3) as xs,
            tc.tile_pool(name="vt", bufs=3) as vtp,
            tc.tile_pool(name="pt", bufs=3) as ptp,
            tc.tile_pool(name="yt", bufs=4) as ytp,
            tc.tile_pool(name="nrm", bufs=3) as nrm,
            tc.tile_pool(name="ob", bufs=3) as ob,
            # PSUM: tag "s" slots are [128, 2, RT] = 2 banks (S^T both heads;
            # projection accumulators use half a slot). bufs=2 -> 4 banks.
            # Tag "y" slots are 1 bank, shared by Y accumulators, out-proj
            # tiles and V-transpose tiles. bufs=4 -> 4 banks. Total 8.
            tc.tile_pool(name="psS", bufs=2, space="PSUM") as psS,
            tc.tile_pool(name="psY", bufs=3, space="PSUM") as psY,
            tc.tile_pool(name="psO", bufs=1, space="PSUM") as psO,
        ):
            # ---- constants ----
            wq = const.tile([128, KT, 128], F32R)
            wk = const.tile([128, KT, 128], F32R)
            wv = const.tile([128, KT, 128], F32R)
            wp = const.tile([128, C], F32R)
            nc.sync.dma_start(wq[:], wq_d.rearrange("(k p) m -> p k m", p=128))
            biases = []
            for name, d in (("bq", bq_d), ("bk", bk_d), ("bv", bv_d)):
                t = const.tile([128, 1], F32, name=f"{name}_sb")
                nc.gpsimd.dma_start(t[:], d.rearrange("(p o) -> p o", o=1))
                biases.append(t)
            Lm = const.tile([128, 128], BF16)
            Em = const.tile([128, 4, RT], BF16)
            ident = const.tile([128, 64], F32R)
            nc.gpsimd.dma_start(ident[:], id_d[:])
            nc.gpsimd.dma_start(Lm[:], L_d[:])
            nc.gpsimd.dma_start(Em[:], E_d[:])
            nc.gpsimd.dma_start(wp[:], wp_d[:])

            # ---- whole-run big buffers (per-rowtile tiles so attention
            # blocks only depend on the projection rowtiles they read) ----
            n_rt_all = rows // RT
            qTs = [big.tile([128, RT], F32R, name=f"qT{i}") for i in range(n_rt_all)]
            kTs = [big.tile([128, RT], F32R, name=f"kT{i}") for i in range(n_rt_all)]
            n_vt = T // 128  # 16 v-tiles per batch per head
            v_aug = big.tile([128, 2, n_vt, 65], F32R)  # per-batch, reused
            nc.gpsimd.dma_start(
                v_aug[:, :, :, 64:65],
                on_d.rearrange("p (h t o) -> p h t o", h=2, o=1),
            )

            x_src = xT_d.rearrange("(k p) r -> p k r", p=128)
            xt0 = xs.tile([128, KT, RT], F32R, name="xt")
            nc.sync.dma_start(xt0[:, 0:2, :], x_src[:, 0:2, 0:RT])
            nc.sync.dma_start(xt0[:, 2:4, :], x_src[:, 2:4, 0:RT])
            nc.sync.dma_start(wk[:], wk_d.rearrange("(k p) m -> p k m", p=128))
            nc.sync.dma_start(xt0[:, 4:6, :], x_src[:, 4:6, 0:RT])
            nc.sync.dma_start(wv[:], wv_d.rearrange("(k p) m -> p k m", p=128))
            nc.sync.dma_start(xt0[:, 6:8, :], x_src[:, 6:8, 0:RT])

            for b in range(n_batches):
                r0 = b * T
                # ---- projections for batch b: 4 row-tiles of 512 ----
                with nc.named_scope(f"proj{b}"):
                    for rt in range(bt_rt):
                        c0 = r0 + rt * RT
                        if b == 0 and rt == 0:
                            xt = xt0
                        else:
                            xt = xs.tile([128, KT, RT], F32R, name="xt")
                            for kh in range(0, KT, 2):
                                nc.sync.dma_start(
                                    xt[:, kh : kh + 2, :],
                                    x_src[:, kh : kh + 2, c0 : c0 + RT],
                                )
                        for w, bias, dest in (
                            (wq, biases[0], qTs[(r0 + rt * RT) // RT]),
                            (wk, biases[1], kTs[(r0 + rt * RT) // RT]),
                            (wv, biases[2], None),
                        ):
                            acc = psS.tile([128, RT], F32, name="proj", tag="s")
                            for k in range(KT):
                                nc.tensor.matmul(
                                    acc[:],
                                    w[:, k, :],
                                    xt[:, k, :],
                                    start=(k == 0),
                                    stop=(k == KT - 1),
                                )
                            if dest is not None:
                                nc.vector.tensor_scalar_add(
                                    dest[:], acc[:], bias[:]
                                )
                            else:
                                vt_sb = vtp.tile([128, RT], F32R, name="vt_sb")
                                nc.vector.tensor_scalar_add(vt_sb[:], acc[:], bias[:])
                                for c in range(RT // 128):
                                    vtile = rt * (RT // 128) + c
                                    vps = psO.tile(
                                        [128, 2, 64], F32R, name="vps", tag="o"
                                    )
                                    for h in range(2):
                                        nc.tensor.transpose(
                                            vps[:, h, :],
                                            vt_sb[
                                                64 * h : 64 * h + 64,
                                                c * 128 : c * 128 + 128,
                                            ],
                                            ident[64 * h : 64 * h + 64, :],
                                        )
                                        nc.vector.tensor_copy(
                                            v_aug[:, h, vtile, 0:64], vps[:, h, :]
                                        )

                # ---- attention for batch b ----
                for tqb in range(bt_rt):
                    with nc.named_scope(f"attn{b}_{tqb}"):
                        q0 = r0 + tqb * RT
                        n_tk = (tqb + 1) * (RT // 128)
                        yps = [
                            psY.tile([65, RT], F32, name=f"yacc{h}", tag="y")
                            for h in range(2)
                        ]
                        for tk in range(n_tk):
                            k0 = r0 + tk * 128
                            diag = tk * 128 >= tqb * RT
                            st = psS.tile([128, 2, RT], F32, name="st", tag="s")
                            kt_tile = kTs[k0 // RT]
                            kk = k0 % RT
                            qt_tile = qTs[q0 // RT]
                            for h in range(2):
                                hs = slice(64 * h, 64 * h + 64)
                                nc.tensor.matmul(
                                    st[:, h, :],
                                    kt_tile[hs, kk : kk + 128],
                                    qt_tile[hs, :],
                                    start=True,
                                    stop=not diag,
                                    skip_group_check=True,
                                )
                            if diag:
                                off_idx = tk - tqb * (RT // 128)
                                for h in range(2):
                                    nc.tensor.matmul(
                                        st[:, h, :],
                                        Lm[:],
                                        Em[:, off_idx, :],
                                        start=False,
                                        stop=True,
                                        skip_group_check=True,
                                    )
                            pt = ptp.tile([128, 2, RT], F32R, name="pt")
                            nc.scalar.activation(
                                pt[:], st[:], mybir.ActivationFunctionType.Exp
                            )
                            for h in range(2):
                                nc.tensor.matmul(
                                    yps[h][:],
                                    v_aug[:, h, tk, :],
                                    pt[:, h, :],
                                    start=(tk == 0),
                                    stop=(tk == n_tk - 1),
                                )
                        # ---- normalize -> Y^T block [128, RT] ----
                        yt = ytp.tile([128, RT], F32R, name="yt")
                        for h in range(2):
                            ssum = nrm.tile([1, RT], F32, name="ssum")
                            nc.vector.tensor_copy(ssum[:], yps[h][64:65, :])
                            srow = nrm.tile([1, RT], F32, name="srow")
                            nc.vector.reciprocal_approx_fast(srow[:], ssum[:])
                            bc = nrm.tile([64, RT], F32, name="bc")
                            nc.gpsimd.partition_broadcast(bc[:], srow[:])
                            nc.vector.tensor_mul(
                                yt[64 * h : 64 * h + 64, :], yps[h][0:64, :], bc[:]
                            )
                    # ---- output projection for this 512-row block ----
                    with nc.named_scope(f"oproj{b}_{tqb}"):
                        for rr in range(RT // 128):
                            for nn in range(C // 512):
                                ops = psO.tile([128, 512], F32, name="ops", tag="o")
                                nc.tensor.matmul(
                                    ops[:],
                                    yt[:, rr * 128 : rr * 128 + 128],
                                    wp[:, nn * 512 : nn * 512 + 512],
                                    start=True,
                                    stop=True,
                                )
                                osb = ob.tile([128, 512], F32, name="osb")
                                nc.vector.tensor_copy(osb[:], ops[:])
                                nc.sync.dma_start(
                                    out_d[
                                        q0 + rr * 128 : q0 + rr * 128 + 128,
                                        nn * 512 : nn * 512 + 512,
                                    ],
                                    osb[:],
                                )
    nc.compile()
    return nc


def make_masks():
    """L/E such that (L.T @ E)[i, j] = NEG iff masked (key i+tk0 > query j+tq0,
    given off = tk0 - tq0 in {0,128,256,384}), else 0."""
    L = np.zeros((128, 128), np.float32)
    for k in range(128):
        L[k, k + 1 :] = NEG
    L[127, :] = NEG
    E = np.zeros((128, 4, RT), np.float32)
    for oi, off in enumerate((0, 128, 256, 384)):
        for k in range(127):
            if k + off < RT:
                E[k, oi, k + off] = 1.0
        E[127, oi, :off] = 1.0
    return L, E


def make_inputs_for_core(c, x, Wq, bq, Wk, bk, Wv, bv, Wp, bp, n_batches=B):
    cols = slice(c * 128, (c + 1) * 128)
    xT = np.ascontiguousarray(
        np.asarray(x, np.float32).reshape(B * T, C)[: n_batches * T].T
    )
    L, E = make_masks()
    idnp = np.zeros((128, 64), np.float32)
    for h in range(2):
        idnp[64 * h : 64 * h + 64] = np.eye(64, dtype=np.float32)
    import ml_dtypes

    return {
        "xT": xT,
        "wq": np.ascontiguousarray(np.asarray(Wq, np.float32)[:, cols] / 8.0),
        "wk": np.ascontiguousarray(np.asarray(Wk, np.float32)[:, cols]),
        "wv": np.ascontiguousarray(np.asarray(Wv, np.float32)[:, cols]),
        "wp": np.ascontiguousarray(np.asarray(Wp, np.float32)[cols, :]),
        "bq": np.ascontiguousarray(np.asarray(bq, np.float32)[cols] / 8.0),
        "bk": np.ascontiguousarray(np.asarray(bk, np.float32)[cols]),
        "bv": np.ascontiguousarray(np.asarray(bv, np.float32)[cols]),
        "Lm": L.astype(ml_dtypes.bfloat16),
        "Em": E.astype(ml_dtypes.bfloat16),
        "ident": idnp,
        "onescol": np.ones((128, 2 * (T // 128)), np.float32),
    }


def kernel(x, Wq, bq, Wk, bk, Wv, bv, Wp, bp, _nc_cache={}, **run_kwargs):
    n_batches = B
    if "nc" not in _nc_cache:
        _nc_cache["nc"] = build_kernel(n_batches)
    nc = _nc_cache["nc"]
    in_maps = [
        make_inputs_for_core(c, x, Wq, bq, Wk, bk, Wv, bv, Wp, bp, n_batches)
        for c in range(N_CORES)
    ]
    res = run_bass_kernel_spmd(nc, in_maps, core_ids=list(range(N_CORES)), **run_kwargs)
    out = np.zeros((B * T, C), np.float32)
    for r in res.results:
        out += r["out"]
    out += np.asarray(bp, np.float32)[None, :]
    if run_kwargs.get("trace"):
        kernel.last_result = res
    return out.reshape(B, T, C)


# revision 17
# speedup vs baseline: 1.1931x; 1.0007x over previous
"""Causal self-attention Trainium2 kernel (8 NeuronCores, tensor-parallel heads).

Problem: B=4, T=2048, C=1024, H=16, D=64 fp32.
  q,k,v = x@Wq+bq, x@Wk+bk, x@Wv+bv  (per-head causal softmax(qk^T/8) @ v) @ Wp + bp

Sharding: 2 heads per core (column-split Wq/Wk/Wv, row-split Wp). Each core
computes a partial output [B*T, C]; host sums the 8 partials and adds bp.

Per-core dataflow (all matmuls in fp32r: full PE rate at N>=512, ~1e-4 rel err):
  xT [C, B*T] fp32 streamed from DRAM (host pre-transposes x).
  Q^T/K^T/V^T [128, B*T] = w.T @ xT  (w slices [C,128] as stationary).
  V natural [tk,64] per head via PE transpose of V^T tiles; ones column
  appended -> V_aug [tk, 65] so P@V_aug also yields softmax row sums.
  S^T [tk,512] = K^T.T @ Q^T per (batch, tq-block, tk-tile); both heads run
  concurrently as K=64 matmuls on distinct PE row groups into one 2-bank
  PSUM tile. Causality is a second accumulated matmul adding -50 on masked
  entries (L/E triangular factorization); softmax then skips the
  max-subtraction pass entirely (scores are O(1); exp stays in fp32 range).
  P^T = exp(S^T) via one double-wide ACT op per slot, PSUM -> SBUF f32r.
  Y_aug^T [65, 512] = V_aug.T @ P^T accumulated over tk tiles; row 64 is the
  softmax denominator. Normalize: DVE copy of the sums row to SBUF ->
  reciprocal_approx_fast -> gpsimd partition_broadcast -> DVE multiply
  -> Y^T [128, RT] per block.
  out_part [512, 1024] = Y^T.T @ Wp_slice per block, DMA'd out as computed.
"""

import numpy as np

import concourse.tile as tile
from concourse import bacc, mybir
from concourse.bass_utils import run_bass_kernel_spmd

F32R = mybir.dt.float32r
F32 = mybir.dt.float32
BF16 = mybir.dt.bfloat16

B, T, C, H = 4, 2048, 1024, 16
D = C // H  # 64
N_CORES = 8
RT = 512  # row-tile (tq block) size
KT = C // 128  # 8 contraction tiles for projections
NEG = -50.0  # causal mask additive constant (exp(-50+s) ~ 1e-20)


def build_kernel(n_batches=B):
    nc = bacc.Bacc(None, target_bir_lowering=False, debug=False)
    rows = n_batches * T
    bt_rt = T // RT  # 4 tq blocks per batch

    xT_d = nc.dram_tensor("xT", [C, rows], F32R, kind="ExternalInput")
    wq_d = nc.dram_tensor("wq", [C, 128], F32R, kind="ExternalInput")
    wk_d = nc.dram_tensor("wk", [C, 128], F32R, kind="ExternalInput")
    wv_d = nc.dram_tensor("wv", [C, 128], F32R, kind="ExternalInput")
    wp_d = nc.dram_tensor("wp", [128, C], F32R, kind="ExternalInput")
    bq_d = nc.dram_tensor("bq", [128], F32, kind="ExternalInput")
    bk_d = nc.dram_tensor("bk", [128], F32, kind="ExternalInput")
    bv_d = nc.dram_tensor("bv", [128], F32, kind="ExternalInput")
    L_d = nc.dram_tensor("Lm", [128, 128], BF16, kind="ExternalInput")
    E_d = nc.dram_tensor("Em", [128, 4, RT], BF16, kind="ExternalInput")
    id_d = nc.dram_tensor("ident", [128, 64], F32R, kind="ExternalInput")
    on_d = nc.dram_tensor("onescol", [128, 2 * (T // 128)], F32R, kind="ExternalInput")
    out_d = nc.dram_tensor("out", [rows, C], F32, kind="ExternalOutput")

    with tile.TileContext(nc) as tc:
        with (
            nc.allow_low_precision(reason="f32r intermediates are intentional"),
            tc.tile_pool(name="const", bufs=1) as const,
            tc.tile_pool(name="big", bufs=1) as big,
            tc.tile_pool(name="xs", bufs=3) as xs,
            tc.tile_pool(name="vt", bufs=3) as vtp,
            tc.tile_pool(name="pt", bufs=3) as ptp,
            tc.tile_pool(name="yt", bufs=4) as ytp,
            tc.tile_pool(name="nrm", bufs=3) as nrm,
            tc.tile_pool(name="ob", bufs=3) as ob,
            # PSUM: tag "s" slots are [128, 2, RT] = 2 banks (S^T both heads;
            # projection accumulators use half a slot). bufs=2 -> 4 banks.
            # Tag "y" slots are 1 bank, shared by Y accumulators, out-proj
            # tiles and V-transpose tiles. bufs=4 -> 4 banks. Total 8.
            tc.tile_pool(name="psS", bufs=2, space="PSUM") as psS,
            tc.tile_pool(name="psY", bufs=3, space="PSUM") as psY,
            tc.tile_pool(name="psO", bufs=1, space="PSUM") as psO,
        ):
            # ---- constants ----
            wq = const.tile([128, KT, 128], F32R)
            wk = const.tile([128, KT, 128], F32R)
            wv = const.tile([128, KT, 128], F32R)
            wp = const.tile([128, C], F32R)
            nc.sync.dma_start(wq[:], wq_d.rearrange("(k p) m -> p k m", p=128))
            biases = []
            for name, d in (("bq", bq_d), ("bk", bk_d), ("bv", bv_d)):
                t = const.tile([128, 1], F32, name=f"{name}_sb")
                nc.gpsimd.dma_start(t[:], d.rearrange("(p o) -> p o", o=1))
                biases.append(t)
            Lm = const.tile([128, 128], BF16)
            Em = const.tile([128, 4, RT], BF16)
            ident = const.tile([128, 64], F32R)
            nc.gpsimd.dma_start(ident[:], id_d[:])
            nc.gpsimd.dma_start(Lm[:], L_d[:])
            nc.gpsimd.dma_start(Em[:], E_d[:])
            nc.gpsimd.dma_start(wp[:], wp_d[:])

            # ---- whole-run big buffers (per-rowtile tiles so attention
            # blocks only depend on the projection rowtiles they read) ----
            n_rt_all = rows // RT
            qTs = [big.tile([128, RT], F32R, name=f"qT{i}") for i in range(n_rt_all)]
            kTs = [big.tile([128, RT], F32R, name=f"kT{i}") for i in range(n_rt_all)]
            n_vt = T // 128  # 16 v-tiles per batch per head
            v_aug = big.tile([128, 2, n_vt, 65], F32R)  # per-batch, reused
            nc.gpsimd.dma_start(
                v_aug[:, :, :, 64:65],
                on_d.rearrange("p (h t o) -> p h t o", h=2, o=1),
            )

            x_src = xT_d.rearrange("(k p) r -> p k r", p=128)
            xt0 = xs.tile([128, KT, RT], F32R, name="xt")
            nc.sync.dma_start(xt0[:, 0:2, :], x_src[:, 0:2, 0:RT])
            nc.sync.dma_start(xt0[:, 2:4, :], x_src[:, 2:4, 0:RT])
            nc.sync.dma_start(wk[:], wk_d.rearrange("(k p) m -> p k m", p=128))
            nc.sync.dma_start(xt0[:, 4:6, :], x_src[:, 4:6, 0:RT])
            nc.sync.dma_start(wv[:], wv_d.rearrange("(k p) m -> p k m", p=128))
            nc.sync.dma_start(xt0[:, 6:8, :], x_src[:, 6:8, 0:RT])

            for b in range(n_batches):
                r0 = b * T
                # ---- projections for batch b: 4 row-tiles of 512 ----
                with nc.named_scope(f"proj{b}"):
                    for rt in range(bt_rt):
                        c0 = r0 + rt * RT
                        if b == 0 and rt == 0:
                            xt = xt0
                        else:
                            xt = xs.tile([128, KT, RT], F32R, name="xt")
                            for kh in range(0, KT, 2):
                                nc.sync.dma_start(
                                    xt[:, kh : kh + 2, :],
                                    x_src[:, kh : kh + 2, c0 : c0 + RT],
                                )
                        for w, bias, dest in (
                            (wq, biases[0], qTs[(r0 + rt * RT) // RT]),
                            (wk, biases[1], kTs[(r0 + rt * RT) // RT]),
                            (wv, biases[2], None),
                        ):
                            acc = psS.tile([128, RT], F32, name="proj", tag="s")
                            for k in range(KT):
                                nc.tensor.matmul(
                                    acc[:],
                                    w[:, k, :],
                                    xt[:, k, :],
                                    start=(k == 0),
                                    stop=(k == KT - 1),
                                )
                            if dest is not None:
                                nc.vector.tensor_scalar_add(
                                    dest[:], acc[:], bias[:]
                                )
                            else:
                                vt_sb = vtp.tile([128, RT], F32R, name="vt_sb")
                                nc.vector.tensor_scalar_add(vt_sb[:], acc[:], bias[:])
                                for c in range(RT // 128):
                                    vtile = rt * (RT // 128) + c
                                    vps = psO.tile(
                                        [128, 2, 64], F32R, name="vps", tag="o"
                                    )
                                    for h in range(2):
                                        nc.tensor.transpose(
                                            vps[:, h, :],
                                            vt_sb[
                                                64 * h : 64 * h + 64,
                                                c * 128 : c * 128 + 128,
                                            ],
                                            ident[64 * h : 64 * h + 64, :],
                                        )
                                        nc.vector.tensor_copy(
                                            v_aug[:, h, vtile, 0:64], vps[:, h, :]
                                        )

                # ---- attention for batch b ----
                for tqb in range(bt_rt):
                    with nc.named_scope(f"attn{b}_{tqb}"):
                        q0 = r0 + tqb * RT
                        n_tk = (tqb + 1) * (RT // 128)
                        yps = [
                            psY.tile([65, RT], F32, name=f"yacc{h}", tag="y")
                            for h in range(2)
                        ]
                        for tk in range(n_tk):
                            k0 = r0 + tk * 128
                            diag = tk * 128 >= tqb * RT
                            st = psS.tile([128, 2, RT], F32, name="st", tag="s")
                            kt_tile = kTs[k0 // RT]
                            kk = k0 % RT
                            qt_tile = qTs[q0 // RT]
                            for h in range(2):
                                hs = slice(64 * h, 64 * h + 64)
                                nc.tensor.matmul(
                                    st[:, h, :],
                                    kt_tile[hs, kk : kk + 128],
                                    qt_tile[hs, :],
                                    start=True,
                                    stop=not diag,
                                    skip_group_check=True,
                                )
                            if diag:
                                off_idx = tk - tqb * (RT // 128)
                                for h in range(2):
                                    nc.tensor.matmul(
                                        st[:, h, :],
                                        Lm[:],
                                        Em[:, off_idx, :],
                                        start=False,
                                        stop=True,
                                        skip_group_check=True,
                                    )
                            pt = ptp.tile([128, 2, RT], F32R, name="pt")
                            nc.scalar.activation(
                                pt[:], st[:], mybir.ActivationFunctionType.Exp
                            )
                            for h in range(2):
                                nc.tensor.matmul(
                                    yps[h][:],
                                    v_aug[:, h, tk, :],
                                    pt[:, h, :],
                                    start=(tk == 0),
                                    stop=(tk == n_tk - 1),
                                )
                        # ---- normalize -> Y^T block [128, RT] ----
                        yt = ytp.tile([128, RT], F32R, name="yt")
                        for h in range(2):
                            ssum = nrm.tile([1, RT], F32, name="ssum")
                            nc.vector.tensor_copy(ssum[:], yps[h][64:65, :])
                            srow = nrm.tile([1, RT], F32, name="srow")
                            nc.vector.reciprocal_approx_fast(srow[:], ssum[:])
                            bc = nrm.tile([64, RT], F32, name="bc")
                            nc.gpsimd.partition_broadcast(bc[:], srow[:])
                            nc.vector.tensor_mul(
                                yt[64 * h : 64 * h + 64, :], yps[h][0:64, :], bc[:]
                            )
                    # ---- output projection for this 512-row block ----
                    with nc.named_scope(f"oproj{b}_{tqb}"):
                        for rr in range(RT // 128):
                            for nn in range(C // 512):
                                ops = psO.tile([128, 512], F32, name="ops", tag="o")
                                nc.tensor.matmul(
                                    ops[:],
                                    yt[:, rr * 128 : rr * 128 + 128],
                                    wp[:, nn * 512 : nn * 512 + 512],
                                    start=True,
                                    stop=True,
                                )
                                osb = ob.tile([128, 512], F32, name="osb")
                                nc.vector.tensor_copy(osb[:], ops[:])
                                nc.sync.dma_start(
                                    out_d[
                                        q0 + rr * 128 : q0 + rr * 128 + 128,
                                        nn * 512 : nn * 512 + 512,
                                    ],
                                    osb[:],
                                )
    nc.compile()
    return nc


def make_masks():
    """L/E such that (L.T @ E)[i, j] = NEG iff masked (key i+tk0 > query j+tq0,
    given off = tk0 - tq0 in {0,128,256,384}), else 0."""
    L = np.zeros((128, 128), np.float32)
    for k in range(128):
        L[k, k + 1 :] = NEG
    L[127, :] = NEG
    E = np.zeros((128, 4, RT), np.float32)
    for oi, off in enumerate((0, 128, 256, 384)):
        for k in range(127):
            if k + off < RT:
                E[k, oi, k + off] = 1.0
        E[127, oi, :off] = 1.0
    return L, E


def make_inputs_for_core(c, x, Wq, bq, Wk, bk, Wv, bv, Wp, bp, n_batches=B):
    cols = slice(c * 128, (c + 1) * 128)
    xT = np.ascontiguousarray(
        np.asarray(x, np.float32).reshape(B * T, C)[: n_batches * T].T
    )
    L, E = make_masks()
    idnp = np.zeros((128, 64), np.float32)
    for h in range(2):
        idnp[64 * h : 64 * h + 64] = np.eye(64, dtype=np.float32)
    import ml_dtypes

    return {
        "xT": xT,
        "wq": np.ascontiguousarray(np.asarray(Wq, np.float32)[:, cols] / 8.0),
        "wk": np.ascontiguousarray(np.asarray(Wk, np.float32)[:, cols]),
        "wv": np.ascontiguousarray(np.asarray(Wv, np.float32)[:, cols]),
        "wp": np.ascontiguousarray(np.asarray(Wp, np.float32)[cols, :]),
        "bq": np.ascontiguousarray(np.asarray(bq, np.float32)[cols] / 8.0),
        "bk": np.ascontiguousarray(np.asarray(bk, np.float32)[cols]),
        "bv": np.ascontiguousarray(np.asarray(bv, np.float32)[cols]),
        "Lm": L.astype(ml_dtypes.bfloat16),
        "Em": E.astype(ml_dtypes.bfloat16),
        "ident": idnp,
        "onescol": np.ones((128, 2 * (T // 128)), np.float32),
    }


def kernel(x, Wq, bq, Wk, bk, Wv, bv, Wp, bp, _nc_cache={}, **run_kwargs):
    n_batches = B
    if "nc" not in _nc_cache:
        _nc_cache["nc"] = build_kernel(n_batches)
    nc = _nc_cache["nc"]
    in_maps = [
        make_inputs_for_core(c, x, Wq, bq, Wk, bk, Wv, bv, Wp, bp, n_batches)
        for c in range(N_CORES)
    ]
    res = run_bass_kernel_spmd(nc, in_maps, core_ids=list(range(N_CORES)), **run_kwargs)
    out = np.zeros((B * T, C), np.float32)
    for r in res.results:
        out += r["out"]
    out += np.asarray(bp, np.float32)[None, :]
    if run_kwargs.get("trace"):
        kernel.last_result = res
    return out.reshape(B, T, C)


# revision 19
# speedup vs baseline: 1.1932x; 1.0000x over previous
"""Causal self-attention Trainium2 kernel (8 NeuronCores, tensor-parallel heads).

Problem: B=4, T=2048, C=1024, H=16, D=64 fp32.
  q,k,v = x@Wq+bq, x@Wk+bk, x@Wv+bv  (per-head causal softmax(qk^T/8) @ v) @ Wp + bp

Sharding: 2 heads per core (column-split Wq/Wk/Wv, row-split Wp). Each core
computes a partial output [B*T, C]; host sums the 8 partials and adds bp.

Per-core dataflow (all matmuls in fp32r: full PE rate at N>=512, ~1e-4 rel err):
  xT [C, B*T] fp32 streamed from DRAM (host pre-transposes x).
  Q^T/K^T/V^T [128, B*T] = w.T @ xT  (w slices [C,128] as stationary).
  V natural [tk,64] per head via PE transpose of V^T tiles; ones column
  appended -> V_aug [tk, 65] so P@V_aug also yields softmax row sums.
  S^T [tk,512] = K^T.T @ Q^T per (batch, tq-block, tk-tile); both heads run
  concurrently as K=64 matmuls on distinct PE row groups into one 2-bank
  PSUM tile. Causality is a second accumulated matmul adding -50 on masked
  entries (L/E triangular factorization); softmax then skips the
  max-subtraction pass entirely (scores are O(1); exp stays in fp32 range).
  P^T = exp(S^T) via one double-wide ACT op per slot, PSUM -> SBUF f32r.
  Y_aug^T [65, 512] = V_aug.T @ P^T accumulated over tk tiles; row 64 is the
  softmax denominator. Normalize: DVE copy of the sums row to SBUF ->
  reciprocal_approx_fast -> gpsimd partition_broadcast -> DVE multiply
  -> Y^T [128, RT] per block.
  out_part [512, 1024] = Y^T.T @ Wp_slice per block, DMA'd out as computed.
"""

import numpy as np

import concourse.tile as tile
from concourse import bacc, mybir
from concourse.bass_utils import run_bass_kernel_spmd

F32R = mybir.dt.float32r
F32 = mybir.dt.float32
BF16 = mybir.dt.bfloat16

B, T, C, H = 4, 2048, 1024, 16
D = C // H  # 64
N_CORES = 8
RT = 512  # row-tile (tq block) size
KT = C // 128  # 8 contraction tiles for projections
NEG = -50.0  # causal mask additive constant (exp(-50+s) ~ 1e-20)


def build_kernel(n_batches=B):
    nc = bacc.Bacc(None, target_bir_lowering=False, debug=False)
    rows = n_batches * T
    bt_rt = T // RT  # 4 tq blocks per batch

    xT_d = nc.dram_tensor("xT", [C, rows], F32R, kind="ExternalInput")
    wq_d = nc.dram_tensor("wq", [C, 128], F32R, kind="ExternalInput")
    wk_d = nc.dram_tensor("wk", [C, 128], F32R, kind="ExternalInput")
    wv_d = nc.dram_tensor("wv", [C, 128], F32R, kind="ExternalInput")
    wp_d = nc.dram_tensor("wp", [128, C], F32R, kind="ExternalInput")
    bq_d = nc.dram_tensor("bq", [128], F32, kind="ExternalInput")
    bk_d = nc.dram_tensor("bk", [128], F32, kind="ExternalInput")
    bv_d = nc.dram_tensor("bv", [128], F32, kind="ExternalInput")
    L_d = nc.dram_tensor("Lm", [128, 128], BF16, kind="ExternalInput")
    E_d = nc.dram_tensor("Em", [128, 4, RT], BF16, kind="ExternalInput")
    id_d = nc.dram_tensor("ident", [128, 64], F32R, kind="ExternalInput")
    on_d = nc.dram_tensor("onescol", [128, 2 * (T // 128)], F32R, kind="ExternalInput")
    out_d = nc.dram_tensor("out", [rows, C], F32, kind="ExternalOutput")

    with tile.TileContext(nc) as tc:
        with (
            nc.allow_low_precision(reason="f32r intermediates are intentional"),
            tc.tile_pool(name="const", bufs=1) as const,
            tc.tile_pool(name="big", bufs=1) as big,
            tc.tile_pool(name="xs", bufs=3) as xs,
            tc.tile_pool(name="vt", bufs=3) as vtp,
            tc.tile_pool(name="pt", bufs=3) as ptp,
            tc.tile_pool(name="yt", bufs=4) as ytp,
            tc.tile_pool(name="nrm", bufs=3) as nrm,
            tc.tile_pool(name="ob", bufs=3) as ob,
            # PSUM: tag "s" slots are [128, 2, RT] = 2 banks (S^T both heads;
            # projection accumulators use half a slot). bufs=2 -> 4 banks.
            # Tag "y" slots are 1 bank, shared by Y accumulators, out-proj
            # tiles and V-transpose tiles. bufs=4 -> 4 banks. Total 8.
            tc.tile_pool(name="psS", bufs=2, space="PSUM") as psS,
            tc.tile_pool(name="psY", bufs=3, space="PSUM") as psY,
            tc.tile_pool(name="psO", bufs=1, space="PSUM") as psO,
        ):
            # ---- constants ----
            wq = const.tile([128, KT, 128], F32R)
            wk = const.tile([128, KT, 128], F32R)
            wv = const.tile([128, KT, 128], F32R)
            wp = const.tile([128, C], F32R)
            nc.sync.dma_start(wq[:], wq_d.rearrange("(k p) m -> p k m", p=128))
            biases = []
            for name, d in (("bq", bq_d), ("bk", bk_d), ("bv", bv_d)):
                t = const.tile([128, 1], F32, name=f"{name}_sb")
                nc.gpsimd.dma_start(t[:], d.rearrange("(p o) -> p o", o=1))
                biases.append(t)
            Lm = const.tile([128, 128], BF16)
            Em = const.tile([128, 4, RT], BF16)
            ident = const.tile([128, 64], F32R)
            nc.gpsimd.dma_start(ident[:], id_d[:])
            nc.gpsimd.dma_start(Lm[:], L_d[:])
            nc.gpsimd.dma_start(Em[:], E_d[:])
            nc.gpsimd.dma_start(wp[:], wp_d[:])

            # ---- whole-run big buffers (per-rowtile tiles so attention
            # blocks only depend on the projection rowtiles they read) ----
            n_rt_all = rows // RT
            qTs = [big.tile([128, RT], F32R, name=f"qT{i}") for i in range(n_rt_all)]
            kTs = [big.tile([128, RT], F32R, name=f"kT{i}") for i in range(n_rt_all)]
            n_vt = T // 128  # 16 v-tiles per batch per head
            v_aug = big.tile([128, 2, n_vt, 65], F32R)  # per-batch, reused
            nc.gpsimd.dma_start(
                v_aug[:, :, :, 64:65],
                on_d.rearrange("p (h t o) -> p h t o", h=2, o=1),
            )

            x_src = xT_d.rearrange("(k p) r -> p k r", p=128)
            xt0 = xs.tile([128, KT, RT], F32R, name="xt")
            nc.sync.dma_start(xt0[:, 0:2, :], x_src[:, 0:2, 0:RT])
            nc.sync.dma_start(xt0[:, 2:4, :], x_src[:, 2:4, 0:RT])
            nc.sync.dma_start(wk[:], wk_d.rearrange("(k p) m -> p k m", p=128))
            nc.sync.dma_start(xt0[:, 4:6, :], x_src[:, 4:6, 0:RT])
            nc.sync.dma_start(wv[:], wv_d.rearrange("(k p) m -> p k m", p=128))
            nc.sync.dma_start(xt0[:, 6:8, :], x_src[:, 6:8, 0:RT])

            for b in range(n_batches):
                r0 = b * T
                # ---- projections for batch b: 4 row-tiles of 512 ----
                with nc.named_scope(f"proj{b}"):
                    for rt in range(bt_rt):
                        c0 = r0 + rt * RT
                        if b == 0 and rt == 0:
                            xt = xt0
                        else:
                            xt = xs.tile([128, KT, RT], F32R, name="xt")
                            for kh in range(0, KT, 2):
                                nc.sync.dma_start(
                                    xt[:, kh : kh + 2, :],
                                    x_src[:, kh : kh + 2, c0 : c0 + RT],
                                )
                        for w, bias, dest in (
                            (wq, biases[0], qTs[(r0 + rt * RT) // RT]),
                            (wk, biases[1], kTs[(r0 + rt * RT) // RT]),
                            (wv, biases[2], None),
                        ):
                            acc = psS.tile([128, RT], F32, name="proj", tag="s")
                            for k in range(KT):
                                nc.tensor.matmul(
                                    acc[:],
                                    w[:, k, :],
                                    xt[:, k, :],
                                    start=(k == 0),
                                    stop=(k == KT - 1),
                                )
                            if dest is not None:
                                nc.vector.tensor_scalar_add(
                                    dest[:], acc[:], bias[:]
                                )
                            else:
                                vt_sb = vtp.tile([128, RT], F32R, name="vt_sb")
                                nc.vector.tensor_scalar_add(vt_sb[:], acc[:], bias[:])
                                for c in range(RT // 128):
                                    vtile = rt * (RT // 128) + c
                                    vps = psO.tile(
                                        [128, 2, 64], F32R, name="vps", tag="o"
                                    )
                                    for h in range(2):
                                        nc.tensor.transpose(
                                            vps[:, h, :],
                                            vt_sb[
                                                64 * h : 64 * h + 64,
                                                c * 128 : c * 128 + 128,
                                            ],
                                            ident[64 * h : 64 * h + 64, :],
                                        )
                                        nc.vector.tensor_copy(
                                            v_aug[:, h, vtile, 0:64], vps[:, h, :]
                                        )

                # ---- attention for batch b ----
                for tqb in range(bt_rt):
                    with nc.named_scope(f"attn{b}_{tqb}"):
                        q0 = r0 + tqb * RT
                        n_tk = (tqb + 1) * (RT // 128)
                        yps = [
                            psY.tile([65, RT], F32, name=f"yacc{h}", tag="y")
                            for h in range(2)
                        ]
                        for tk in range(n_tk):
                            k0 = r0 + tk * 128
                            diag = tk * 128 >= tqb * RT
                            st = psS.tile([128, 2, RT], F32, name="st", tag="s")
                            kt_tile = kTs[k0 // RT]
                            kk = k0 % RT
                            qt_tile = qTs[q0 // RT]
                            for h in range(2):
                                hs = slice(64 * h, 64 * h + 64)
                                nc.tensor.matmul(
                                    st[:, h, :],
                                    kt_tile[hs, kk : kk + 128],
                                    qt_tile[hs, :],
                                    start=True,
                                    stop=not diag,
                                    skip_group_check=True,
                                )
                            if diag:
                                off_idx = tk - tqb * (RT // 128)
                                for h in range(2):
                                    nc.tensor.matmul(
                                        st[:, h, :],
                                        Lm[:],
                                        Em[:, off_idx, :],
                                        start=False,
                                        stop=True,
                                        skip_group_check=True,
                                    )
                            pt = ptp.tile([128, 2, RT], F32R, name="pt")
                            nc.scalar.activation(
                                pt[:], st[:], mybir.ActivationFunctionType.Exp
                            )
                            for h in range(2):
                                nc.tensor.matmul(
                                    yps[h][:],
                                    v_aug[:, h, tk, :],
                                    pt[:, h, :],
                                    start=(tk == 0),
                                    stop=(tk == n_tk - 1),
                                )
                        # ---- normalize -> Y^T block [128, RT] ----
                        yt = ytp.tile([128, RT], F32R, name="yt")
                        for h in range(2):
                            ssum = nrm.tile([1, RT], F32, name="ssum")
                            nc.vector.tensor_copy(ssum[:], yps[h][64:65, :])
                            srow = nrm.tile([1, RT], F32, name="srow")
                            nc.vector.reciprocal_approx_fast(srow[:], ssum[:])
                            bc = nrm.tile([64, RT], F32, name="bc")
                            nc.gpsimd.partition_broadcast(bc[:], srow[:])
                            nc.vector.tensor_mul(
                                yt[64 * h : 64 * h + 64, :], yps[h][0:64, :], bc[:]
                            )
                    # ---- output projection for this 512-row block ----
                    with nc.named_scope(f"oproj{b}_{tqb}"):
                        for rr in range(RT // 128):
                            for nn in range(C // 512):
                                ops = psO.tile([128, 512], F32, name="ops", tag="o")
                                nc.tensor.matmul(
                                    ops[:],
                                    yt[:, rr * 128 : rr * 128 + 128],
                                    wp[:, nn * 512 : nn * 512 + 512],
                                    start=True,
                                    stop=True,
                                )
                                osb = ob.tile([128, 512], F32, name="osb")
                                nc.vector.tensor_copy(osb[:], ops[:])
                                nc.sync.dma_start(
                                    out_d[
                                        q0 + rr * 128 : q0 + rr * 128 + 128,
                                        nn * 512 : nn * 512 + 512,
                                    ],
                                    osb[:],
                                )
    nc.compile()
    return nc


def make_masks():
    """L/E such that (L.T @ E)[i, j] = NEG iff masked (key i+tk0 > query j+tq0,
    given off = tk0 - tq0 in {0,128,256,384}), else 0."""
    L = np.zeros((128, 128), np.float32)
    for k in range(128):
        L[k, k + 1 :] = NEG
    L[127, :] = NEG
    E = np.zeros((128, 4, RT), np.float32)
    for oi, off in enumerate((0, 128, 256, 384)):
        for k in range(127):
            if k + off < RT:
                E[k, oi, k + off] = 1.0
        E[127, oi, :off] = 1.0
    return L, E


def make_inputs_for_core(c, x, Wq, bq, Wk, bk, Wv, bv, Wp, bp, n_batches=B):
    cols = slice(c * 128, (c + 1) * 128)
    xT = np.ascontiguousarray(
        np.asarray(x, np.float32).reshape(B * T, C)[: n_batches * T].T
    )
    L, E = make_masks()
    idnp = np.zeros((128, 64), np.float32)
    for h in range(2):
        idnp[64 * h : 64 * h + 64] = np.eye(64, dtype=np.float32)
    import ml_dtypes

    return {
        "xT": xT,
        "wq": np.ascontiguousarray(np.asarray(Wq, np.float32)[:, cols] / 8.0),
        "wk": np.ascontiguousarray(np.asarray(Wk, np.float32)[:, cols]),
        "wv": np.ascontiguousarray(np.asarray(Wv, np.float32)[:, cols]),
        "wp": np.ascontiguousarray(np.asarray(Wp, np.float32)[cols, :]),
        "bq": np.ascontiguousarray(np.asarray(bq, np.float32)[cols] / 8.0),
        "bk": np.ascontiguousarray(np.asarray(bk, np.float32)[cols]),
        "bv": np.ascontiguousarray(np.asarray(bv, np.float32)[cols]),
        "Lm": L.astype(ml_dtypes.bfloat16),
        "Em": E.astype(ml_dtypes.bfloat16),
        "ident": idnp,
        "onescol": np.ones((128, 2 * (T // 128)), np.float32),
    }


def kernel(x, Wq, bq, Wk, bk, Wv, bv, Wp, bp, _nc_cache={}, **run_kwargs):
    n_batches = B
    if "nc" not in _nc_cache:
        _nc_cache["nc"] = build_kernel(n_batches)
    nc = _nc_cache["nc"]
    in_maps = [
        make_inputs_for_core(c, x, Wq, bq, Wk, bk, Wv, bv, Wp, bp, n_batches)
        for c in range(N_CORES)
    ]
    res = run_bass_kernel_spmd(nc, in_maps, core_ids=list(range(N_CORES)), **run_kwargs)
    out = np.zeros((B * T, C), np.float32)
    for r in res.results:
        out += r["out"]
    out += np.asarray(bp, np.float32)[None, :]
    if run_kwargs.get("trace"):
        kernel.last_result = res
    return out.reshape(B, T, C)


# revision 20
# speedup vs baseline: 1.2065x; 1.0112x over previous
"""Causal self-attention Trainium2 kernel (8 NeuronCores, tensor-parallel heads).

Problem: B=4, T=2048, C=1024, H=16, D=64 fp32.
  q,k,v = x@Wq+bq, x@Wk+bk, x@Wv+bv  (per-head causal softmax(qk^T/8) @ v) @ Wp + bp

Sharding: 2 heads per core (column-split Wq/Wk/Wv, row-split Wp). Each core
computes a partial output [B*T, C]; host sums the 8 partials and adds bp.

Per-core dataflow (all matmuls in fp32r: full PE rate at N>=512, ~1e-4 rel err):
  xT [C, B*T] fp32 streamed from DRAM (host pre-transposes x).
  Q^T/K^T/V^T [128, B*T] = w.T @ xT  (w slices [C,128] as stationary).
  V natural [tk,64] per head via PE transpose of V^T tiles; ones column
  appended -> V_aug [tk, 65] so P@V_aug also yields softmax row sums.
  S^T [tk,512] = K^T.T @ Q^T per (batch, tq-block, tk-tile); both heads run
  concurrently as K=64 matmuls on distinct PE row groups into one 2-bank
  PSUM tile. Causality is a second accumulated matmul adding -50 on masked
  entries (L/E triangular factorization); softmax then skips the
  max-subtraction pass entirely (scores are O(1); exp stays in fp32 range).
  P^T = exp(S^T) via one double-wide ACT op per slot, PSUM -> SBUF f32r.
  Y_aug^T [65, 512] = V_aug.T @ P^T accumulated over tk tiles; row 64 is the
  softmax denominator. Normalize: DVE copy of the sums row to SBUF ->
  reciprocal_approx_fast -> gpsimd partition_broadcast -> DVE multiply
  -> Y^T [128, RT] per block.
  out_part [512, 1024] = Y^T.T @ Wp_slice per block, DMA'd out as computed.
"""

import numpy as np

import concourse.tile as tile
from concourse import bacc, mybir
from concourse.bass_utils import run_bass_kernel_spmd

F32R = mybir.dt.float32r
F32 = mybir.dt.float32
BF16 = mybir.dt.bfloat16

B, T, C, H = 4, 2048, 1024, 16
D = C // H  # 64
N_CORES = 8
RT = 512  # row-tile (tq block) size
KT = C // 128  # 8 contraction tiles for projections
NEG = -50.0  # causal mask additive constant (exp(-50+s) ~ 1e-20)


def build_kernel(n_batches=B):
    nc = bacc.Bacc(None, target_bir_lowering=False, debug=False)
    rows = n_batches * T
    bt_rt = T // RT  # 4 tq blocks per batch

    xT_d = nc.dram_tensor("xT", [C, rows], F32R, kind="ExternalInput")
    wq_d = nc.dram_tensor("wq", [C, 128], F32R, kind="ExternalInput")
    wk_d = nc.dram_tensor("wk", [C, 128], F32R, kind="ExternalInput")
    wv_d = nc.dram_tensor("wv", [C, 128], F32R, kind="ExternalInput")
    wp_d = nc.dram_tensor("wp", [128, C], F32R, kind="ExternalInput")
    bq_d = nc.dram_tensor("bq", [128], F32, kind="ExternalInput")
    bk_d = nc.dram_tensor("bk", [128], F32, kind="ExternalInput")
    bv_d = nc.dram_tensor("bv", [128], F32, kind="ExternalInput")
    L_d = nc.dram_tensor("Lm", [128, 128], BF16, kind="ExternalInput")
    E_d = nc.dram_tensor("Em", [128, 4, RT], BF16, kind="ExternalInput")
    id_d = nc.dram_tensor("ident", [128, 64], F32R, kind="ExternalInput")
    on_d = nc.dram_tensor("onescol", [128, 2 * (T // 128)], F32R, kind="ExternalInput")
    out_d = nc.dram_tensor("out", [rows, C], F32, kind="ExternalOutput")

    with tile.TileContext(nc) as tc:
        with (
            nc.allow_low_precision(reason="f32r intermediates are intentional"),
            tc.tile_pool(name="const", bufs=1) as const,
            tc.tile_pool(name="big", bufs=1) as big,
            tc.tile_pool(name="xs", bufs=3) as xs,
            tc.tile_pool(name="vt", bufs=3) as vtp,
            tc.tile_pool(name="pt", bufs=3) as ptp,
            tc.tile_pool(name="yt", bufs=4) as ytp,
            tc.tile_pool(name="nrm", bufs=3) as nrm,
            tc.tile_pool(name="ob", bufs=3) as ob,
            # PSUM: tag "s" slots are [128, 2, RT] = 2 banks (S^T both heads;
            # projection accumulators use half a slot). bufs=2 -> 4 banks.
            # Tag "y" slots are 1 bank, shared by Y accumulators, out-proj
            # tiles and V-transpose tiles. bufs=4 -> 4 banks. Total 8.
            tc.tile_pool(name="psS", bufs=2, space="PSUM") as psS,
            tc.tile_pool(name="psY", bufs=3, space="PSUM") as psY,
            tc.tile_pool(name="psO", bufs=1, space="PSUM") as psO,
        ):
            # ---- constants ----
            wq = const.tile([128, KT, 128], F32R)
            wk = const.tile([128, KT, 128], F32R)
            wv = const.tile([128, KT, 128], F32R)
            wp = const.tile([128, C], F32R)
            nc.sync.dma_start(wq[:], wq_d.rearrange("(k p) m -> p k m", p=128))
            biases = []
            for name, d in (("bq", bq_d), ("bk", bk_d), ("bv", bv_d)):
                t = const.tile([128, 1], F32, name=f"{name}_sb")
                nc.gpsimd.dma_start(t[:], d.rearrange("(p o) -> p o", o=1))
                biases.append(t)
            Lm = const.tile([128, 128], BF16)
            Em = const.tile([128, 4, RT], BF16)
            ident = const.tile([128, 64], F32R)
            nc.gpsimd.dma_start(ident[:], id_d[:])
            nc.gpsimd.dma_start(Lm[:], L_d[:])
            nc.gpsimd.dma_start(Em[:], E_d[:])
            nc.gpsimd.dma_start(wp[:], wp_d[:])

            # ---- whole-run big buffers (per-rowtile tiles so attention
            # blocks only depend on the projection rowtiles they read) ----
            n_rt_all = rows // RT
            qTs = [big.tile([128, RT], F32R, name=f"qT{i}") for i in range(n_rt_all)]
            kTs = [big.tile([128, RT], F32R, name=f"kT{i}") for i in range(n_rt_all)]
            n_vt = T // 128  # 16 v-tiles per batch per head
            v_aug = big.tile([128, 2, n_vt, 65], F32R)  # per-batch, reused
            nc.gpsimd.dma_start(
                v_aug[:, :, :, 64:65],
                on_d.rearrange("p (h t o) -> p h t o", h=2, o=1),
            )

            x_src = xT_d.rearrange("(k p) r -> p k r", p=128)
            xt0 = xs.tile([128, KT, RT], F32R, name="xt")
            nc.sync.dma_start(xt0[:, 0:2, :], x_src[:, 0:2, 0:RT])
            nc.sync.dma_start(xt0[:, 2:4, :], x_src[:, 2:4, 0:RT])
            nc.sync.dma_start(wk[:], wk_d.rearrange("(k p) m -> p k m", p=128))
            nc.sync.dma_start(xt0[:, 4:6, :], x_src[:, 4:6, 0:RT])
            nc.sync.dma_start(wv[:], wv_d.rearrange("(k p) m -> p k m", p=128))
            nc.sync.dma_start(xt0[:, 6:8, :], x_src[:, 6:8, 0:RT])

            pending_oproj = []

            def emit_oproj(yt, q0):
                for rr in range(RT // 128):
                    for nn in range(C // 512):
                        ops = psO.tile([128, 512], F32, name="ops", tag="o")
                        nc.tensor.matmul(
                            ops[:],
                            yt[:, rr * 128 : rr * 128 + 128],
                            wp[:, nn * 512 : nn * 512 + 512],
                            start=True,
                            stop=True,
                        )
                        osb = ob.tile([128, 512], F32, name="osb")
                        nc.vector.tensor_copy(osb[:], ops[:])
                        nc.sync.dma_start(
                            out_d[
                                q0 + rr * 128 : q0 + rr * 128 + 128,
                                nn * 512 : nn * 512 + 512,
                            ],
                            osb[:],
                        )

            for b in range(n_batches):
                r0 = b * T
                # ---- projections for batch b: 4 row-tiles of 512 ----
                with nc.named_scope(f"proj{b}"):
                    for rt in range(bt_rt):
                        c0 = r0 + rt * RT
                        if b == 0 and rt == 0:
                            xt = xt0
                        else:
                            xt = xs.tile([128, KT, RT], F32R, name="xt")
                            for kh in range(0, KT, 2):
                                nc.sync.dma_start(
                                    xt[:, kh : kh + 2, :],
                                    x_src[:, kh : kh + 2, c0 : c0 + RT],
                                )
                        for w, bias, dest in (
                            (wq, biases[0], qTs[(r0 + rt * RT) // RT]),
                            (wk, biases[1], kTs[(r0 + rt * RT) // RT]),
                            (wv, biases[2], None),
                        ):
                            acc = psS.tile([128, RT], F32, name="proj", tag="s")
                            for k in range(KT):
                                nc.tensor.matmul(
                                    acc[:],
                                    w[:, k, :],
                                    xt[:, k, :],
                                    start=(k == 0),
                                    stop=(k == KT - 1),
                                )
                            if dest is not None:
                                nc.vector.tensor_scalar_add(
                                    dest[:], acc[:], bias[:]
                                )
                            else:
                                vt_sb = vtp.tile([128, RT], F32R, name="vt_sb")
                                nc.vector.tensor_scalar_add(vt_sb[:], acc[:], bias[:])
                                for c in range(RT // 128):
                                    vtile = rt * (RT // 128) + c
                                    vps = psY.tile(
                                        [128, 2, 64], F32R, name="vps", tag="y"
                                    )
                                    for h in range(2):
                                        nc.tensor.transpose(
                                            vps[:, h, :],
                                            vt_sb[
                                                64 * h : 64 * h + 64,
                                                c * 128 : c * 128 + 128,
                                            ],
                                            ident[64 * h : 64 * h + 64, :],
                                        )
                                        nc.vector.tensor_copy(
                                            v_aug[:, h, vtile, 0:64], vps[:, h, :]
                                        )

                # ---- attention for batch b ----
                for tqb in range(bt_rt):
                    with nc.named_scope(f"attn{b}_{tqb}"):
                        if pending_oproj:
                            emit_oproj(*pending_oproj.pop())
                        q0 = r0 + tqb * RT
                        n_tk = (tqb + 1) * (RT // 128)
                        yps = [
                            psY.tile([65, RT], F32, name=f"yacc{h}", tag="y")
                            for h in range(2)
                        ]
                        for tk in range(n_tk):
                            k0 = r0 + tk * 128
                            diag = tk * 128 >= tqb * RT
                            st = psS.tile([128, 2, RT], F32, name="st", tag="s")
                            kt_tile = kTs[k0 // RT]
                            kk = k0 % RT
                            qt_tile = qTs[q0 // RT]
                            for h in range(2):
                                hs = slice(64 * h, 64 * h + 64)
                                nc.tensor.matmul(
                                    st[:, h, :],
                                    kt_tile[hs, kk : kk + 128],
                                    qt_tile[hs, :],
                                    start=True,
                                    stop=not diag,
                                    skip_group_check=True,
                                )
                            if diag:
                                off_idx = tk - tqb * (RT // 128)
                                for h in range(2):
                                    nc.tensor.matmul(
                                        st[:, h, :],
                                        Lm[:],
                                        Em[:, off_idx, :],
                                        start=False,
                                        stop=True,
                                        skip_group_check=True,
                                    )
                            pt = ptp.tile([128, 2, RT], F32R, name="pt")
                            nc.scalar.activation(
                                pt[:], st[:], mybir.ActivationFunctionType.Exp
                            )
                            for h in range(2):
                                nc.tensor.matmul(
                                    yps[h][:],
                                    v_aug[:, h, tk, :],
                                    pt[:, h, :],
                                    start=(tk == 0),
                                    stop=(tk == n_tk - 1),
                                )
                        # ---- normalize -> Y^T block [128, RT] ----
                        yt = ytp.tile([128, RT], F32R, name="yt")
                        for h in range(2):
                            ssum = nrm.tile([1, RT], F32, name="ssum")
                            nc.vector.tensor_copy(ssum[:], yps[h][64:65, :])
                            srow = nrm.tile([1, RT], F32, name="srow")
                            nc.vector.reciprocal_approx_fast(srow[:], ssum[:])
                            bc = nrm.tile([64, RT], F32, name="bc")
                            nc.gpsimd.partition_broadcast(bc[:], srow[:])
                            nc.vector.tensor_mul(
                                yt[64 * h : 64 * h + 64, :], yps[h][0:64, :], bc[:]
                            )
                    pending_oproj.append((yt, q0))
            while pending_oproj:
                emit_oproj(*pending_oproj.pop())
    nc.compile()
    return nc


def make_masks():
    """L/E such that (L.T @ E)[i, j] = NEG iff masked (key i+tk0 > query j+tq0,
    given off = tk0 - tq0 in {0,128,256,384}), else 0."""
    L = np.zeros((128, 128), np.float32)
    for k in range(128):
        L[k, k + 1 :] = NEG
    L[127, :] = NEG
    E = np.zeros((128, 4, RT), np.float32)
    for oi, off in enumerate((0, 128, 256, 384)):
        for k in range(127):
            if k + off < RT:
                E[k, oi, k + off] = 1.0
        E[127, oi, :off] = 1.0
    return L, E


def make_inputs_for_core(c, x, Wq, bq, Wk, bk, Wv, bv, Wp, bp, n_batches=B):
    cols = slice(c * 128, (c + 1) * 128)
    xT = np.ascontiguousarray(
        np.asarray(x, np.float32).reshape(B * T, C)[: n_batches * T].T
    )
    L, E = make_masks()
    idnp = np.zeros((128, 64), np.float32)
    for h in range(2):
        idnp[64 * h : 64 * h + 64] = np.eye(64, dtype=np.float32)
    import ml_dtypes

    return {
        "xT": xT,
        "wq": np.ascontiguousarray(np.asarray(Wq, np.float32)[:, cols] / 8.0),
        "wk": np.ascontiguousarray(np.asarray(Wk, np.float32)[:, cols]),
        "wv": np.ascontiguousarray(np.asarray(Wv, np.float32)[:, cols]),
        "wp": np.ascontiguousarray(np.asarray(Wp, np.float32)[cols, :]),
        "bq": np.ascontiguousarray(np.asarray(bq, np.float32)[cols] / 8.0),
        "bk": np.ascontiguousarray(np.asarray(bk, np.float32)[cols]),
        "bv": np.ascontiguousarray(np.asarray(bv, np.float32)[cols]),
        "Lm": L.astype(ml_dtypes.bfloat16),
        "Em": E.astype(ml_dtypes.bfloat16),
        "ident": idnp,
        "onescol": np.ones((128, 2 * (T // 128)), np.float32),
    }


def kernel(x, Wq, bq, Wk, bk, Wv, bv, Wp, bp, _nc_cache={}, **run_kwargs):
    n_batches = B
    if "nc" not in _nc_cache:
        _nc_cache["nc"] = build_kernel(n_batches)
    nc = _nc_cache["nc"]
    in_maps = [
        make_inputs_for_core(c, x, Wq, bq, Wk, bk, Wv, bv, Wp, bp, n_batches)
        for c in range(N_CORES)
    ]
    res = run_bass_kernel_spmd(nc, in_maps, core_ids=list(range(N_CORES)), **run_kwargs)
    out = np.zeros((B * T, C), np.float32)
    for r in res.results:
        out += r["out"]
    out += np.asarray(bp, np.float32)[None, :]
    if run_kwargs.get("trace"):
        kernel.last_result = res
    return out.reshape(B, T, C)


# revision 21
# speedup vs baseline: 1.2141x; 1.0063x over previous
"""Causal self-attention Trainium2 kernel (8 NeuronCores, tensor-parallel heads).

Problem: B=4, T=2048, C=1024, H=16, D=64 fp32.
  q,k,v = x@Wq+bq, x@Wk+bk, x@Wv+bv  (per-head causal softmax(qk^T/8) @ v) @ Wp + bp

Sharding: 2 heads per core (column-split Wq/Wk/Wv, row-split Wp). Each core
computes a partial output [B*T, C]; host sums the 8 partials and adds bp.

Per-core dataflow (all matmuls in fp32r: full PE rate at N>=512, ~1e-4 rel err):
  xT [C, B*T] fp32 streamed from DRAM (host pre-transposes x).
  Q^T/K^T/V^T [128, B*T] = w.T @ xT  (w slices [C,128] as stationary).
  V natural [tk,64] per head via PE transpose of V^T tiles; ones column
  appended -> V_aug [tk, 65] so P@V_aug also yields softmax row sums.
  S^T [tk,512] = K^T.T @ Q^T per (batch, tq-block, tk-tile); both heads run
  concurrently as K=64 matmuls on distinct PE row groups into one 2-bank
  PSUM tile. Causality is a second accumulated matmul adding -50 on masked
  entries (L/E triangular factorization); softmax then skips the
  max-subtraction pass entirely (scores are O(1); exp stays in fp32 range).
  P^T = exp(S^T) via one double-wide ACT op per slot, PSUM -> SBUF f32r.
  Y_aug^T [65, 512] = V_aug.T @ P^T accumulated over tk tiles; row 64 is the
  softmax denominator. Normalize: DVE copy of the sums row to SBUF ->
  reciprocal_approx_fast -> gpsimd partition_broadcast -> DVE multiply
  -> Y^T [128, RT] per block.
  out_part [512, 1024] = Y^T.T @ Wp_slice per block, DMA'd out as computed.
"""

import numpy as np

import concourse.tile as tile
from concourse import bacc, mybir
from concourse.bass_utils import run_bass_kernel_spmd

F32R = mybir.dt.float32r
F32 = mybir.dt.float32
BF16 = mybir.dt.bfloat16

B, T, C, H = 4, 2048, 1024, 16
D = C // H  # 64
N_CORES = 8
RT = 512  # row-tile (tq block) size
KT = C // 128  # 8 contraction tiles for projections
NEG = -50.0  # causal mask additive constant (exp(-50+s) ~ 1e-20)


def build_kernel(n_batches=B):
    nc = bacc.Bacc(None, target_bir_lowering=False, debug=False)
    rows = n_batches * T
    bt_rt = T // RT  # 4 tq blocks per batch

    xT_d = nc.dram_tensor("xT", [C, rows], F32R, kind="ExternalInput")
    wq_d = nc.dram_tensor("wq", [C, 128], F32R, kind="ExternalInput")
    wk_d = nc.dram_tensor("wk", [C, 128], F32R, kind="ExternalInput")
    wv_d = nc.dram_tensor("wv", [C, 128], F32R, kind="ExternalInput")
    wp_d = nc.dram_tensor("wp", [128, C], F32R, kind="ExternalInput")
    bq_d = nc.dram_tensor("bq", [128], F32, kind="ExternalInput")
    bk_d = nc.dram_tensor("bk", [128], F32, kind="ExternalInput")
    bv_d = nc.dram_tensor("bv", [128], F32, kind="ExternalInput")
    L_d = nc.dram_tensor("Lm", [128, 128], BF16, kind="ExternalInput")
    E_d = nc.dram_tensor("Em", [128, 4, RT], BF16, kind="ExternalInput")
    id_d = nc.dram_tensor("ident", [128, 64], F32R, kind="ExternalInput")
    on_d = nc.dram_tensor("onescol", [128, 2 * (T // 128)], F32R, kind="ExternalInput")
    out_d = nc.dram_tensor("out", [rows, C], F32, kind="ExternalOutput")

    with tile.TileContext(nc) as tc:
        with (
            nc.allow_low_precision(reason="f32r intermediates are intentional"),
            tc.tile_pool(name="const", bufs=1) as const,
            tc.tile_pool(name="big", bufs=1) as big,
            tc.tile_pool(name="xs", bufs=3) as xs,
            tc.tile_pool(name="vt", bufs=2) as vtp,
            tc.tile_pool(name="pt", bufs=4) as ptp,
            tc.tile_pool(name="yt", bufs=4) as ytp,
            tc.tile_pool(name="nrm", bufs=3) as nrm,
            tc.tile_pool(name="ob", bufs=2) as ob,
            # PSUM: tag "s" slots are [128, 2, RT] = 2 banks (S^T both heads;
            # projection accumulators use half a slot). bufs=2 -> 4 banks.
            # Tag "y" slots are 1 bank, shared by Y accumulators, out-proj
            # tiles and V-transpose tiles. bufs=4 -> 4 banks. Total 8.
            tc.tile_pool(name="psS", bufs=2, space="PSUM") as psS,
            tc.tile_pool(name="psY", bufs=3, space="PSUM") as psY,
            tc.tile_pool(name="psO", bufs=1, space="PSUM") as psO,
        ):
            # ---- constants ----
            wq = const.tile([128, KT, 128], F32R)
            wk = const.tile([128, KT, 128], F32R)
            wv = const.tile([128, KT, 128], F32R)
            wp = const.tile([128, C], F32R)
            nc.sync.dma_start(wq[:], wq_d.rearrange("(k p) m -> p k m", p=128))
            biases = []
            for name, d in (("bq", bq_d), ("bk", bk_d), ("bv", bv_d)):
                t = const.tile([128, 1], F32, name=f"{name}_sb")
                nc.gpsimd.dma_start(t[:], d.rearrange("(p o) -> p o", o=1))
                biases.append(t)
            Lm = const.tile([128, 128], BF16)
            Em = const.tile([128, 4, RT], BF16)
            ident = const.tile([128, 64], F32R)
            nc.gpsimd.dma_start(ident[:], id_d[:])
            nc.gpsimd.dma_start(Lm[:], L_d[:])
            nc.gpsimd.dma_start(Em[:], E_d[:])
            nc.gpsimd.dma_start(wp[:], wp_d[:])

            # ---- whole-run big buffers (per-rowtile tiles so attention
            # blocks only depend on the projection rowtiles they read) ----
            n_rt_all = rows // RT
            qTs = [big.tile([128, RT], F32R, name=f"qT{i}") for i in range(n_rt_all)]
            kTs = [big.tile([128, RT], F32R, name=f"kT{i}") for i in range(n_rt_all)]
            n_vt = T // 128  # 16 v-tiles per batch per head
            v_aug = big.tile([128, 2, n_vt, 65], F32R)  # per-batch, reused
            nc.gpsimd.dma_start(
                v_aug[:, :, :, 64:65],
                on_d.rearrange("p (h t o) -> p h t o", h=2, o=1),
            )

            x_src = xT_d.rearrange("(k p) r -> p k r", p=128)
            xt0 = xs.tile([128, KT, RT], F32R, name="xt")
            nc.sync.dma_start(xt0[:, 0:2, :], x_src[:, 0:2, 0:RT])
            nc.sync.dma_start(xt0[:, 2:4, :], x_src[:, 2:4, 0:RT])
            nc.sync.dma_start(wk[:], wk_d.rearrange("(k p) m -> p k m", p=128))
            nc.sync.dma_start(xt0[:, 4:6, :], x_src[:, 4:6, 0:RT])
            nc.sync.dma_start(wv[:], wv_d.rearrange("(k p) m -> p k m", p=128))
            nc.sync.dma_start(xt0[:, 6:8, :], x_src[:, 6:8, 0:RT])

            pending_oproj = []

            def emit_oproj(yt, q0, final=False):
                for rr in range(RT // 128):
                    for nn in range(C // 512):
                        if final and (rr * 2 + nn) % 2 == 1:
                            ops = psS.tile([128, 512], F32, name="opsf", tag="s")
                        else:
                            ops = psO.tile([128, 512], F32, name="ops", tag="o")
                        nc.tensor.matmul(
                            ops[:],
                            yt[:, rr * 128 : rr * 128 + 128],
                            wp[:, nn * 512 : nn * 512 + 512],
                            start=True,
                            stop=True,
                        )
                        osb = ob.tile([128, 512], F32, name="osb")
                        nc.vector.tensor_copy(osb[:], ops[:])
                        nc.sync.dma_start(
                            out_d[
                                q0 + rr * 128 : q0 + rr * 128 + 128,
                                nn * 512 : nn * 512 + 512,
                            ],
                            osb[:],
                        )

            for b in range(n_batches):
                r0 = b * T
                # ---- projections for batch b: 4 row-tiles of 512 ----
                with nc.named_scope(f"proj{b}"):
                    for rt in range(bt_rt):
                        c0 = r0 + rt * RT
                        if b == 0 and rt == 0:
                            xt = xt0
                        else:
                            xt = xs.tile([128, KT, RT], F32R, name="xt")
                            for kh in range(0, KT, 2):
                                nc.sync.dma_start(
                                    xt[:, kh : kh + 2, :],
                                    x_src[:, kh : kh + 2, c0 : c0 + RT],
                                )
                        for w, bias, dest in (
                            (wq, biases[0], qTs[(r0 + rt * RT) // RT]),
                            (wk, biases[1], kTs[(r0 + rt * RT) // RT]),
                            (wv, biases[2], None),
                        ):
                            acc = psS.tile([128, RT], F32, name="proj", tag="s")
                            for k in range(KT):
                                nc.tensor.matmul(
                                    acc[:],
                                    w[:, k, :],
                                    xt[:, k, :],
                                    start=(k == 0),
                                    stop=(k == KT - 1),
                                )
                            if dest is not None:
                                nc.vector.tensor_scalar_add(
                                    dest[:], acc[:], bias[:]
                                )
                            else:
                                vt_sb = vtp.tile([128, RT], F32R, name="vt_sb")
                                nc.vector.tensor_scalar_add(vt_sb[:], acc[:], bias[:])
                                for c in range(RT // 128):
                                    vtile = rt * (RT // 128) + c
                                    vps = psY.tile(
                                        [128, 2, 64], F32R, name="vps", tag="y"
                                    )
                                    for h in range(2):
                                        nc.tensor.transpose(
                                            vps[:, h, :],
                                            vt_sb[
                                                64 * h : 64 * h + 64,
                                                c * 128 : c * 128 + 128,
                                            ],
                                            ident[64 * h : 64 * h + 64, :],
                                        )
                                        nc.vector.tensor_copy(
                                            v_aug[:, h, vtile, 0:64], vps[:, h, :]
                                        )

                # ---- attention for batch b ----
                for tqb in range(bt_rt):
                    with nc.named_scope(f"attn{b}_{tqb}"):
                        if pending_oproj:
                            emit_oproj(*pending_oproj.pop())
                        q0 = r0 + tqb * RT
                        n_tk = (tqb + 1) * (RT // 128)
                        yps = [
                            psY.tile([65, RT], F32, name=f"yacc{h}", tag="y")
                            for h in range(2)
                        ]
                        for tk in range(n_tk):
                            k0 = r0 + tk * 128
                            diag = tk * 128 >= tqb * RT
                            st = psS.tile([128, 2, RT], F32, name="st", tag="s")
                            kt_tile = kTs[k0 // RT]
                            kk = k0 % RT
                            qt_tile = qTs[q0 // RT]
                            for h in range(2):
                                hs = slice(64 * h, 64 * h + 64)
                                nc.tensor.matmul(
                                    st[:, h, :],
                                    kt_tile[hs, kk : kk + 128],
                                    qt_tile[hs, :],
                                    start=True,
                                    stop=not diag,
                                    skip_group_check=True,
                                )
                            if diag:
                                off_idx = tk - tqb * (RT // 128)
                                for h in range(2):
                                    nc.tensor.matmul(
                                        st[:, h, :],
                                        Lm[:],
                                        Em[:, off_idx, :],
                                        start=False,
                                        stop=True,
                                        skip_group_check=True,
                                    )
                            pt = ptp.tile([128, 2, RT], F32R, name="pt")
                            nc.scalar.activation(
                                pt[:], st[:], mybir.ActivationFunctionType.Exp
                            )
                            for h in range(2):
                                nc.tensor.matmul(
                                    yps[h][:],
                                    v_aug[:, h, tk, :],
                                    pt[:, h, :],
                                    start=(tk == 0),
                                    stop=(tk == n_tk - 1),
                                )
                        # ---- normalize -> Y^T block [128, RT] ----
                        yt = ytp.tile([128, RT], F32R, name="yt")
                        for h in range(2):
                            ssum = nrm.tile([1, RT], F32, name="ssum")
                            nc.vector.tensor_copy(ssum[:], yps[h][64:65, :])
                            srow = nrm.tile([1, RT], F32, name="srow")
                            nc.vector.reciprocal_approx_fast(srow[:], ssum[:])
                            bc = nrm.tile([64, RT], F32, name="bc")
                            nc.gpsimd.partition_broadcast(bc[:], srow[:])
                            nc.vector.tensor_mul(
                                yt[64 * h : 64 * h + 64, :], yps[h][0:64, :], bc[:]
                            )
                    pending_oproj.append((yt, q0))
            while pending_oproj:
                emit_oproj(*pending_oproj.pop(), final=True)
    nc.compile()
    return nc


def make_masks():
    """L/E such that (L.T @ E)[i, j] = NEG iff masked (key i+tk0 > query j+tq0,
    given off = tk0 - tq0 in {0,128,256,384}), else 0."""
    L = np.zeros((128, 128), np.float32)
    for k in range(128):
        L[k, k + 1 :] = NEG
    L[127, :] = NEG
    E = np.zeros((128, 4, RT), np.float32)
    for oi, off in enumerate((0, 128, 256, 384)):
        for k in range(127):
            if k + off < RT:
                E[k, oi, k + off] = 1.0
        E[127, oi, :off] = 1.0
    return L, E


def make_inputs_for_core(c, x, Wq, bq, Wk, bk, Wv, bv, Wp, bp, n_batches=B):
    cols = slice(c * 128, (c + 1) * 128)
    xT = np.ascontiguousarray(
        np.asarray(x, np.float32).reshape(B * T, C)[: n_batches * T].T
    )
    L, E = make_masks()
    idnp = np.zeros((128, 64), np.float32)
    for h in range(2):
        idnp[64 * h : 64 * h + 64] = np.eye(64, dtype=np.float32)
    import ml_dtypes

    return {
        "xT": xT,
        "wq": np.ascontiguousarray(np.asarray(Wq, np.float32)[:, cols] / 8.0),
        "wk": np.ascontiguousarray(np.asarray(Wk, np.float32)[:, cols]),
        "wv": np.ascontiguousarray(np.asarray(Wv, np.float32)[:, cols]),
        "wp": np.ascontiguousarray(np.asarray(Wp, np.float32)[cols, :]),
        "bq": np.ascontiguousarray(np.asarray(bq, np.float32)[cols] / 8.0),
        "bk": np.ascontiguousarray(np.asarray(bk, np.float32)[cols]),
        "bv": np.ascontiguousarray(np.asarray(bv, np.float32)[cols]),
        "Lm": L.astype(ml_dtypes.bfloat16),
        "Em": E.astype(ml_dtypes.bfloat16),
        "ident": idnp,
        "onescol": np.ones((128, 2 * (T // 128)), np.float32),
    }


def kernel(x, Wq, bq, Wk, bk, Wv, bv, Wp, bp, _nc_cache={}, **run_kwargs):
    n_batches = B
    if "nc" not in _nc_cache:
        _nc_cache["nc"] = build_kernel(n_batches)
    nc = _nc_cache["nc"]
    in_maps = [
        make_inputs_for_core(c, x, Wq, bq, Wk, bk, Wv, bv, Wp, bp, n_batches)
        for c in range(N_CORES)
    ]
    res = run_bass_kernel_spmd(nc, in_maps, core_ids=list(range(N_CORES)), **run_kwargs)
    out = np.zeros((B * T, C), np.float32)
    for r in res.results:
        out += r["out"]
    out += np.asarray(bp, np.float32)[None, :]
    if run_kwargs.get("trace"):
        kernel.last_result = res
    return out.reshape(B, T, C)
